# revision 4
# baseline (speedup 1.0000x reference)
"""ResNet bottleneck block (dense_cnn) on 8 Trainium2 NeuronCores.

Reference computation (NCHW, fp32):
    t1  = relu(s1 * conv1x1(x, w1, stride=2) + b1)     # 512 -> 256, 28x28 -> 14x14
    t2  = relu(s2 * conv3x3(t1, w2, pad=1)   + b2)     # 256 -> 256
    t3  =      s3 * conv1x1(t2, w3)          + b3      # 256 -> 1024
    idn =      s4 * conv1x1(x, w4, stride=2) + b4      # 512 -> 1024
    out = relu(t3 + idn)                               # (64, 1024, 14, 14)

Strategy:
  - Data-parallel over batch: 64 images -> 8 cores x 8 images.
  - Host-side prep (numpy, cheap): subsample x to its even (h, w)
    positions, fold BN scales into conv weights, transpose weights to
    [ci, co], cast to bf16 (rel err ~5e-3 << the 2e-2 gate at ~2x the
    PE/DMA throughput of fp32 paths).
  - On-chip: every conv is a matmul with channels on partitions and
    (image, h, w) on the free dim (392 columns = 2 images).  The 3x3
    conv is 9 shifted matmuls accumulating in PSUM over zero-padded
    16-wide planes of t1.  Weight-stationary ordering amortizes
    LDWEIGHTS (hidden by the PE reorder window).
  - PSUM: each half-stage owns a [128, 2048]-f32 tile (4 banks); the 4
    image groups live at 512-col bank-aligned offsets so a whole
    m-group drains with ONE wide ACT/DVE instruction (bias+relu fused)
    instead of 4-16 narrow ones; engine fixed overhead (~0.2us/instr)
    amortizes 4-8x.
  - Stage 1 is k-outer (both conv1 output halves accumulate in PSUM
    across arriving xs chunks) so each 0.4MB xs chunk unlocks 8
    matmuls and the input stream stays ahead of the PE.
  - Stage 3 issues the conv4 (residual, xs-only) matmuls before conv3
    so the PE fills while stage 2 still drains; conv3 accumulates into
    the same PSUM so the add + final relu are one pass.
  - DMA queues: SP-HWDGE carries w1+xs(+biases) -- a pure-dma_start
    instruction stream, so the next For_i iteration's loads prefetch as
    soon as buffers free instead of queueing behind drain work.
    ACT-HWDGE carries w2/w3/w4 (not needed until ~10us in) and the last
    two output chunks; the Pool-engine SWDGE streams the rest of the
    output.  t1pad zeroing only touches the halo border (~900 cols vs
    4352 for full planes).
  - Post-finalize IR passes: drop InstLdweights duplicated by
    legalization for repeated stationary operands, thin matmul
    semaphore updates to the waited counts, and strip semaphore waits
    already implied by earlier waits on the same engine.

Measured per-execution device time via a hardware For_i loop slope:
~66us on the axon-tunneled trn2 (baseline of this session: ~80us).
Pure-PE floor for 368 matmuls of 392 cols at the measured back-to-back
rate (24ns + 0.52ns/col under full-power P0 clocks) is ~64us.
"""

import os

import numpy as np

import concourse.mybir as mybir
import concourse.tile as tile
from concourse import bacc
from concourse.bass_utils import run_bass_kernel_spmd

F32 = mybir.dt.float32
BF16 = mybir.dt.bfloat16
F32R = mybir.dt.float32r
I32 = mybir.dt.int32

N_CORES = 8
B = 8              # images per core
HW = 14            # output spatial
P = HW * HW        # 196 per image plane (compact)
PB = B * P         # 1568
WP = 16            # padded row width for the 3x3 conv input
Q = HW * WP        # 224 (padded-plane columns per image in conv2 psum)
PADQ = 17 * WP     # 272 per-image padded plane (1 extra slack row)
NG = 2             # images per matmul group
G = B // NG        # 4 groups
NF = NG * P        # 392: compact moving-operand free size
GP = 512           # psum columns per image group (bank-aligned)

# Compute dtype for matmuls: "f32r" (fp32 storage, TF32-like multiply,
# full PE rate), "f32" (exact, 1/4 rate), "bf16".
COMPUTE_DT = os.environ.get("BOT_DT", "bf16")
# Debug: build only the first N stages (1..3) for per-stage HW timing.
STAGES = int(os.environ.get("BOT_STAGES", "3"))

_CACHE = {}


def _dedupe_ldweights(nc):
    """Drop InstLdweights identical to the previous one in the PE stream.

    Legalization emits one weight load per matmul; when consecutive
    matmuls share the stationary operand the repeated ~107ns loads are
    pure PE overhead.  Only waits/updates-free duplicates are dropped,
    so semaphore counts are unchanged.
    """

    def ap_sig(ap):
        try:
            ml = ap.memorylocation
            name = ml.name if ml is not None else None
        except Exception:
            name = None
        off = getattr(ap, "offset", None)
        try:
            dims = tuple((d.num_elem, d.step) for d in ap.aps)
        except Exception:
            dims = str(ap)
        return (name, off, dims)

    removed = 0
    for fn in nc.m.functions:
        for blk in fn.blocks:
            insts = blk.instructions
            last_sig = None
            keep = []
            for inst in insts:
                if isinstance(inst, mybir.InstLdweights):
                    sig = (
                        ap_sig(inst.ins[0]),
                        getattr(inst, "perf_mode", None),
                        getattr(inst, "is_transpose", None),
                        getattr(inst, "tile_position", None),
                    )
                    si = inst.sync_info
                    clean = si is None or (
                        len(si.on_wait) == 0 and len(si.on_update) == 0
                    )
                    if clean and sig == last_sig:
                        removed += 1
                        continue
                    last_sig = sig
                elif not isinstance(inst, mybir.InstMatmult):
                    if isinstance(
                        inst,
                        (mybir.InstUnconditionalBranch, mybir.InstCall),
                    ):
                        last_sig = None
                keep.append(inst)
            if len(keep) != len(insts):
                del insts[:]
                insts.extend(keep)
    return removed


def _strip_redundant_waits(nc):
    """Remove semaphore waits already implied by earlier waits.

    Engines execute their instruction stream in order, so once an
    instruction on engine E has waited for sem >= v, every later wait on
    E for sem >= v' with v' <= v is a no-op.  Each retired wait still
    costs the sequencer dispatch time, so stripping them shortens the
    per-instruction issue path.  Tracking is per block and resets at
    event-semaphore (barrier/reset) instructions, which is conservative
    for loop back-edges.
    """
    removed = 0
    for fn in nc.m.functions:
        for blk in fn.blocks:
            seen = {}  # (engine, sem_id) -> max value waited
            for inst in blk.instructions:
                if isinstance(inst, mybir.InstEventSemaphore):
                    seen = {k: v for k, v in seen.items() if k[0] != inst.engine}
                    # barriers also imply cross-engine sync; be safe:
                    seen = {}
                    continue
                si = inst.sync_info
                if si is None or not si.on_wait:
                    continue
                kept = []
                for w in si.on_wait:
                    if (
                        getattr(w, "sync_type", None) == "semaphore"
                        and getattr(w, "wait_mode", None) == "sem-ge-imm"
                        and getattr(w, "wait_value", None) is not None
                    ):
                        key = (inst.engine, w.id)
                        if seen.get(key, -1) >= w.wait_value:
                            removed += 1
                            continue
                        seen[key] = w.wait_value
                    kept.append(w)
                if len(kept) != len(si.on_wait):
                    si.on_wait = kept
    return removed


def _thin_pe_sem_updates(nc):
    """Drop matmul semaphore increments no consumer distinguishes.

    Every matmul increments the PE progress semaphore (~26ns serialized
    EVT_SEM write each), but consumers wait on only a few distinct
    counts.  Keep exactly the increments at waited cumulative counts and
    renumber every wait to its rank among kept values.  Applied only to
    semaphores whose updates are exclusively matmul sem-inc(+1) and
    whose waits are all sem-ge-imm, so semantics are preserved.
    """
    removed = 0
    for fn in nc.m.functions:
        upd, bad, waits = {}, set(), {}
        for blk in fn.blocks:
            for inst in blk.instructions:
                si = inst.sync_info
                if not si:
                    continue
                for u in si.on_update:
                    if getattr(u, "sync_type", None) != "semaphore":
                        continue
                    if (
                        getattr(u, "update_mode", None) != "sem-inc"
                        or getattr(u, "update_value", None) != 1
                        or not isinstance(inst, mybir.InstMatmult)
                    ):
                        bad.add(u.id)
                    upd.setdefault(u.id, []).append((inst, u))
                for w in si.on_wait:
                    if getattr(w, "sync_type", None) != "semaphore":
                        continue
                    if (
                        getattr(w, "wait_mode", None) != "sem-ge-imm"
                        or getattr(w, "wait_value", None) is None
                    ):
                        bad.add(getattr(w, "id", None))
                        continue
                    waits.setdefault(w.id, []).append(w)
        for sid, entries in upd.items():
            if sid in bad or len(entries) < 8:
                continue
            vals = sorted({w.wait_value for w in waits.get(sid, [])})
            pos = [v for v in vals if v >= 1]
            if not pos or pos[-1] > len(entries):
                continue
            keep = set(pos)
            rank = {v: i + 1 for i, v in enumerate(pos)}
            for idx, (inst, u) in enumerate(entries):
                if idx + 1 not in keep:
                    inst.sync_info.on_update = [
                        x for x in inst.sync_info.on_update if x is not u
                    ]
                    removed += 1
            for w in waits.get(sid, []):
                if w.wait_value >= 1:
                    w.wait_value = rank[w.wait_value]
    return removed


def _build_nc(reps=1, loop_n=0):
    """loop_n > 0 wraps the body in a hardware For_i loop (timing only)."""
    act_dt = {"bf16": BF16, "f32": F32, "f32r": F32R}[COMPUTE_DT]
    # bf16 output halves the out DMA and the drain write traffic; host
    # upcasts after gather.  Error stays ~1e-3 << the 2e-2 gate.
    out_dt = BF16 if (act_dt == BF16
                      and os.environ.get("BOT_OUT_BF16", "1") == "1") else F32

    nc = bacc.Bacc()
    xs_d = nc.declare_dram_parameter("xs", [512, PB], act_dt, isOutput=False)
    w1_d = nc.declare_dram_parameter("w1t", [512, 256], act_dt, isOutput=False)
    w2_d = nc.declare_dram_parameter("w2t", [9 * 256, 256], act_dt, isOutput=False)
    w3_d = nc.declare_dram_parameter("w3t", [256, 1024], act_dt, isOutput=False)
    w4_d = nc.declare_dram_parameter("w4t", [512, 1024], act_dt, isOutput=False)
    b1_d = nc.declare_dram_parameter("b1p", [128, 2], F32, isOutput=False)
    b2_d = nc.declare_dram_parameter("b2p", [128, 2], F32, isOutput=False)
    b34_d = nc.declare_dram_parameter("b34p", [128, 8], F32, isOutput=False)
    out_d = nc.declare_dram_parameter("out", [1024, PB], out_dt, isOutput=True)

    relu = mybir.ActivationFunctionType.Relu
    alu_add = mybir.AluOpType.add
    alu_max = mybir.AluOpType.max

    # stage-1/2 m-halves whose drain runs on ACT (rest on DVE)
    s12_act = {
        int(x)
        for x in os.environ.get("BOT_S12_ACT", "0").split(",")
        if x != ""
    }
    # stage-3 m-groups whose drain runs on ACT (rest on DVE)
    s3_act = {
        int(x)
        for x in os.environ.get("BOT_S3_ACT", "1,5").split(",")
        if x != ""
    }
    # stage-3 m-groups whose output DMA rides the ACT HWDGE queue
    out_act = {
        int(x)
        for x in os.environ.get("BOT_OUT_ACT", "6,7").split(",")
        if x != ""
    }

    def post(on_act, dst, src, bias_ap):
        # relu(src + bias) -> dst on the chosen engine
        if on_act:
            nc.scalar.activation(dst, src, relu, bias=bias_ap)
        else:
            nc.vector.tensor_scalar(dst, src, bias_ap, 0.0, alu_add, alu_max)

    import contextlib

    with tile.TileContext(nc) as tc:
        with (
            tc.tile_pool(name="consts", bufs=2) as consts,
            tc.tile_pool(name="psum", bufs=2, space="PSUM") as psum,
            tc.tile_pool(name="outp", bufs=3) as outp,
            (
                tc.For_i(0, loop_n, 1, hint_engines=(mybir.EngineType.PE,),
                         staggered_reset=True)
                if loop_n
                else contextlib.nullcontext()
            ),
        ):
            for _rep in range(reps):
                # --- SP queue: w1 first (unblocks the PE), then the xs
                # chunks in consumption order, then the small biases.
                # A pure-dma_start stream: next iteration's loads issue
                # as soon as pool buffers free.
                w1_t = consts.tile([128, 4 * 256], act_dt, tag="w1")
                nc.sync.dma_start(
                    out=w1_t.rearrange("p (k c) -> p k c", k=4),
                    in_=w1_d.rearrange("(k p) c -> p k c", p=128),
                )
                w1_sb = [w1_t[:, k * 256:(k + 1) * 256] for k in range(4)]
                b1_sb = consts.tile([128, 2], F32, tag="b1")
                nc.sync.dma_start(out=b1_sb, in_=b1_d[:, :])
                xs_sb = [
                    consts.tile([128, PB], act_dt, tag=f"xs_{k}", name=f"xs_{k}")
                    for k in range(4)
                ]
                for k in range(4):
                    nc.sync.dma_start(
                        out=xs_sb[k], in_=xs_d[k * 128:(k + 1) * 128, :]
                    )
                b2_sb = consts.tile([128, 2], F32, tag="b2")
                nc.sync.dma_start(out=b2_sb, in_=b2_d[:, :])
                b34_sb = consts.tile([128, 8], F32, tag="b34")
                nc.sync.dma_start(out=b34_sb, in_=b34_d[:, :])

                # --- ACT queue: w2/w3/w4, needed from ~10us in ---
                w2_t = consts.tile([128, 18 * 256], act_dt, tag="w2")
                nc.scalar.dma_start(
                    out=w2_t.rearrange("p (c n) -> p c n", c=18),
                    in_=w2_d.rearrange("(c p) n -> p c n", p=128),
                )
                w2_sb = [
                    [
                        w2_t[:, (tap * 2 + k) * 256:(tap * 2 + k + 1) * 256]
                        for k in range(2)
                    ]
                    for tap in range(9)
                ]
                w3_t = consts.tile([128, 2 * 1024], act_dt, tag="w3")
                nc.scalar.dma_start(
                    out=w3_t.rearrange("p (k c) -> p k c", k=2),
                    in_=w3_d.rearrange("(k p) c -> p k c", p=128),
                )
                w3_sb = [w3_t[:, k * 1024:(k + 1) * 1024] for k in range(2)]
                w4_t = consts.tile([128, 4 * 1024], act_dt, tag="w4")
                nc.scalar.dma_start(
                    out=w4_t.rearrange("p (k c) -> p k c", k=4),
                    in_=w4_d.rearrange("(k p) c -> p k c", p=128),
                )
                w4_sb = [w4_t[:, k * 1024:(k + 1) * 1024] for k in range(4)]

                # --- t1 padded planes: zero only the halo border ---
                t1pad = []
                for k in range(2):
                    t = consts.tile([128, B * PADQ], act_dt, tag=f"t1p_{k}")
                    if os.environ.get("BOT_MEMSET", "border") == "full":
                        nc.vector.memset(t, 0.0)
                    else:
                        pl = t.rearrange("p (i q) -> p i q", i=B)
                        # rows 15..16 of every image
                        nc.vector.memset(pl[:, :, 15 * WP:], 0.0)
                        # row 0 of every image
                        nc.vector.memset(pl[:, :, 0:WP], 0.0)
                        # cols 15,0-of-next-row for rows 1..14
                        nc.vector.memset(
                            pl.rearrange("p i (h w) -> p i h w", w=WP)[
                                :, :, 1:15, 15:16
                            ],
                            0.0,
                        )
                        nc.vector.memset(
                            pl.rearrange("p i (h w) -> p i h w", w=WP)[
                                :, :, 1:15, 0:1
                            ],
                            0.0,
                        )
                    t1pad.append(t)
                t2_sb = []
                for k in range(2):
                    t = consts.tile([128, PB], act_dt, tag=f"t2_{k}")
                    t2_sb.append(t)

                # --- stage 1: conv1 + relu, scattered into padded planes.
                # k-outer: each arriving xs chunk feeds 8 matmuls (both
                # output halves), PSUM accumulates across chunks.
                if STAGES >= 1:
                    ps1 = [
                        psum.tile([128, 4 * GP], F32, tag="ps", name=f"s1m{m}")
                        for m in range(2)
                    ]
                    for k in range(4):
                        for m in range(2):
                            for g in range(G):
                                nc.tensor.matmul(
                                    ps1[m][:, g * GP:g * GP + NF],
                                    w1_sb[k][:, m * 128:(m + 1) * 128],
                                    xs_sb[k][:, g * NF:(g + 1) * NF],
                                    start=(k == 0),
                                    stop=(k == 3),
                                )
                    for m in range(2):
                        for g in range(G):
                            src = ps1[m][:, g * GP:g * GP + NF]
                            dst = t1pad[m][
                                :, g * NG * PADQ:(g * NG + NG) * PADQ
                            ].rearrange("p (n h w) -> p n h w", h=17, w=WP)[
                                :, :, 1:15, 1:15
                            ]
                            post(m in s12_act, dst, src, b1_sb[:, m:m + 1])

                # --- stage 2: conv2 (3x3 as 9 shifted matmuls) + relu ---
                # weight-stationary: each tap/k weight feeds all 4 image
                # groups; moving operand is a 4-level shifted view of the
                # padded planes so pad columns are never streamed
                for m in range(2 if STAGES >= 2 else 0):
                    pst = psum.tile([128, 4 * GP], F32, tag="ps", name=f"s2m{m}")
                    i = 0
                    for tap in range(9):
                        dy, dx = divmod(tap, 3)
                        for k in range(2):
                            for g in range(G):
                                seg = t1pad[k][
                                    :, g * NG * PADQ:(g * NG + NG) * PADQ
                                ].rearrange(
                                    "p (n h w) -> p n h w", h=17, w=WP
                                )[:, :, dy:dy + HW, dx:dx + HW]
                                nc.tensor.matmul(
                                    pst[:, g * GP:g * GP + NF],
                                    w2_sb[tap][k][:, m * 128:(m + 1) * 128],
                                    seg,
                                    start=(i == 0),
                                    stop=(i == 17),
                                )
                            i += 1
                    # one wide drain per m: [p, 4, 392] -> t2 contiguous
                    src = pst.rearrange("p (g c) -> p g c", g=4)[:, :, 0:NF]
                    dst = t2_sb[m].rearrange("p (g c) -> p g c", g=4)
                    post(m in s12_act, dst, src, b2_sb[:, m:m + 1])

                # --- stage 3: conv3 + residual conv4 in one PSUM, relu ---
                for m in range(8 if STAGES >= 3 else 0):
                    pst = psum.tile([128, 4 * GP], F32, tag="ps", name=f"s3m{m}")
                    # conv4 first: it depends only on xs, so its matmuls can
                    # fill the PE while stage 2 still drains
                    for k in range(4):
                        for g in range(G):
                            nc.tensor.matmul(
                                pst[:, g * GP:g * GP + NF],
                                w4_sb[k][:, m * 128:(m + 1) * 128],
                                xs_sb[k][:, g * NF:(g + 1) * NF],
                                start=(k == 0),
                                stop=False,
                            )
                    for k in range(2):
                        for g in range(G):
                            nc.tensor.matmul(
                                pst[:, g * GP:g * GP + NF],
                                w3_sb[k][:, m * 128:(m + 1) * 128],
                                t2_sb[k][:, g * NF:(g + 1) * NF],
                                start=False,
                                stop=(k == 1),
                            )
                    ot = outp.tile([128, PB], out_dt, tag="ot")
                    src = pst.rearrange("p (g c) -> p g c", g=4)[:, :, 0:NF]
                    dst = ot.rearrange("p (g c) -> p g c", g=4)
                    post(m in s3_act, dst, src, b34_sb[:, m:m + 1])
                    if m in out_act:
                        nc.scalar.dma_start(
                            out=out_d[m * 128:(m + 1) * 128, :], in_=ot,
                        )
                    else:
                        nc.gpsimd.dma_start(
                            out=out_d[m * 128:(m + 1) * 128, :], in_=ot,
                        )
                if STAGES < 3:
                    ot = outp.tile([128, PB], out_dt, tag="ot")
                    src_t = (t2_sb[0] if STAGES >= 2 else
                             (t1pad[0][:, 0:PB] if STAGES >= 1
                              else xs_sb[0][:, 0:PB]))
                    nc.vector.tensor_copy(ot, src_t)
                    nc.gpsimd.dma_start(out=out_d[0:128, :], in_=ot)
    nc.finalize()
    if os.environ.get("BOT_LDW_DEDUP", "1") == "1":
        n = _dedupe_ldweights(nc)
        if os.environ.get("BOT_VERBOSE"):
            print(f"[kernel] deduped {n} InstLdweights")
    if os.environ.get("BOT_SEM_THIN", "1") == "1":
        n = _thin_pe_sem_updates(nc)
        if os.environ.get("BOT_VERBOSE"):
            print(f"[kernel] thinned {n} matmul sem updates")
    if os.environ.get("BOT_WAIT_STRIP", "1") == "1":
        n = _strip_redundant_waits(nc)
        if os.environ.get("BOT_VERBOSE"):
            print(f"[kernel] stripped {n} redundant waits")
    return nc


def _prep(x, w1, w2, w3, w4, s1, b1, s2, b2, s3, b3, s4, b4):
    """Host-side input prep: shard, fold BN, transpose. All numpy."""
    if COMPUTE_DT == "bf16":
        import ml_dtypes

        cdt = np.dtype(ml_dtypes.bfloat16)
    else:
        cdt = np.dtype(np.float32)

    # x -> even positions, (core, c, n, h*14+w) channel-major partition lines
    xs = x[:, :, ::2, ::2].reshape(N_CORES, B, 512, P).transpose(0, 2, 1, 3)
    xs = np.ascontiguousarray(xs).reshape(N_CORES, 512, PB).astype(cdt)

    w1f = (w1[:, :, 0, 0] * s1[:, None]).T                    # (512, 256)
    w2f = w2 * s2[:, None, None, None]                        # (256,256,3,3)
    w2t = np.stack(
        [w2f[:, :, dy, dx].T for dy in range(3) for dx in range(3)]
    ).reshape(9 * 256, 256)                                   # (2304, 256)
    w3f = (w3[:, :, 0, 0] * s3[:, None]).T                    # (256, 1024)
    w4f = (w4[:, :, 0, 0] * s4[:, None]).T                    # (512, 1024)

    com = {
        "w1t": np.ascontiguousarray(w1f).astype(cdt),
        "w2t": np.ascontiguousarray(w2t).astype(cdt),
        "w3t": np.ascontiguousarray(w3f).astype(cdt),
        "w4t": np.ascontiguousarray(w4f).astype(cdt),
        "b1p": np.ascontiguousarray(b1.reshape(2, 128).T).astype(np.float32),
        "b2p": np.ascontiguousarray(b2.reshape(2, 128).T).astype(np.float32),
        "b34p": np.ascontiguousarray(
            (b3 + b4).reshape(8, 128).T
        ).astype(np.float32),
    }
    return [{"xs": xs[c], **com} for c in range(N_CORES)]


def _gather(results):
    out = np.empty((64, 1024, HW, HW), np.float32)
    for c, r in enumerate(results):
        o = r["out"].astype(np.float32).reshape(1024, B, HW, HW)
        out[c * B:(c + 1) * B] = o.transpose(1, 0, 2, 3)
    return out


def _get_nc(reps=1, loop_n=0):
    key = ("nc", reps, loop_n)
    if key not in _CACHE:
        _CACHE[key] = _build_nc(reps, loop_n)
    return _CACHE[key]


def _run(in_maps, **kwargs):
    return run_bass_kernel_spmd(
        _get_nc(), in_maps, list(range(N_CORES)), **kwargs
    )


def kernel(**inputs):
    in_maps = _prep(**inputs)
    res = _run(in_maps)
    return _gather(res.results)


def _pjrt_runner(nc, in_maps):
    """Compile nc once; return (run_once, run_batch, results).

    run_once(): one blocking execution. run_batch(n): n pipelined
    executions, blocking at the end; returns elapsed seconds. results:
    first run's outputs as a list of per-core dicts.
    """
    import time

    import jax
    import numpy as np_
    from jax.sharding import Mesh, NamedSharding, PartitionSpec
    from jax.experimental.shard_map import shard_map

    from concourse import bass2jax, mybir as mb

    bass2jax.install_neuronx_cc_hook()
    part_name = nc.partition_id_tensor.name if nc.partition_id_tensor else None
    in_names, out_names, out_avals = [], [], []
    for alloc in nc.m.functions[0].allocations:
        if not isinstance(alloc, mb.MemoryLocationSet):
            continue
        name = alloc.memorylocations[0].name
        if alloc.kind == "ExternalInput":
            if name != part_name:
                in_names.append(name)
        elif alloc.kind == "ExternalOutput":
            out_names.append(name)
            out_avals.append(
                jax.core.ShapedArray(
                    tuple(alloc.tensor_shape), mb.dt.np(alloc.dtype)
                )
            )
    all_names = in_names + out_names + ([part_name] if part_name else [])

    def _body(*args):
        operands = list(args)
        if part_name is not None:
            operands.append(bass2jax.partition_id_tensor())
        outs = bass2jax._bass_exec_p.bind(
            *operands,
            out_avals=tuple(out_avals),
            in_names=tuple(all_names),
            out_names=tuple(out_names),
            lowering_input_output_aliases=(),
            sim_require_finite=False,
            sim_require_nnan=False,
            nc=nc,
        )
        return tuple(outs)

    devices = jax.devices()[:N_CORES]
    mesh = Mesh(np_.asarray(devices), ("core",))
    nspec = len(in_names) + len(out_names)
    sharded = jax.jit(
        shard_map(
            _body,
            mesh=mesh,
            in_specs=(PartitionSpec("core"),) * nspec,
            out_specs=(PartitionSpec("core"),) * len(out_names),
            check_rep=False,
        ),
        keep_unused=True,
    )

    sh = NamedSharding(mesh, PartitionSpec("core"))
    dev_args = [
        jax.device_put(
            np_.concatenate([in_maps[c][n] for c in range(N_CORES)], axis=0), sh
        )
        for n in in_names
    ] + [
        jax.device_put(
            np_.zeros((N_CORES * a.shape[0], *a.shape[1:]), a.dtype), sh
        )
        for a in out_avals
    ]

    outs = jax.block_until_ready(sharded(*dev_args))  # compile + warm

    results = [
        {
            n: np_.asarray(outs[i]).reshape(N_CORES, *out_avals[i].shape)[c]
            for i, n in enumerate(out_names)
        }
        for c in range(N_CORES)
    ]

    def run_once():
        jax.block_until_ready(sharded(*dev_args))

    def run_batch(n):
        t0 = time.monotonic()
        r = None
        for _ in range(n):
            r = sharded(*dev_args)
        jax.block_until_ready(r)
        return time.monotonic() - t0

    return run_once, run_batch, results


def kernel_timed(**inputs):
    """Run + estimate steady-state per-execution device time (ns).

    NTFF profiling is unavailable under this axon client and per-call
    wall time is ~80ms of tunnel overhead, so device time is measured
    with a hardware For_i loop: two NEFFs run the 2-rep body loop_n1 /
    loop_n2 times; the wall-time slope over (loop_n2-loop_n1)*2 reps
    cancels the per-call overhead.  Returns (out, exec_time_ns).
    """
    import time

    import numpy as np_

    n1 = int(os.environ.get("BOT_LOOP_N1", "32"))
    n2 = int(os.environ.get("BOT_LOOP_N2", "224"))
    trials = int(os.environ.get("BOT_LOOP_TRIALS", "30"))
    in_maps = _prep(**inputs)

    res = _run(in_maps)
    out = _gather(res.results)

    once1, _, _ = _pjrt_runner(_get_nc(2, loop_n=n1), in_maps)
    once2, _, _ = _pjrt_runner(_get_nc(2, loop_n=n2), in_maps)

    def s(f):
        t0 = time.monotonic()
        f()
        return time.monotonic() - t0

    for _ in range(2):
        s(once1), s(once2)
    t1s, t2s = [], []
    for _ in range(trials):
        t1s.append(s(once1))
        t2s.append(s(once2))
    t1s, t2s = np_.array(t1s), np_.array(t2s)
    per = (t2s - t1s) / (n2 - n1) / 2 * 1e9
    per_rep = int(np_.median(per))
    print(f"[bench] For_i loop slope over {trials} trials: {per_rep} ns "
          f"(iqr {np_.percentile(per, 25):.0f}-{np_.percentile(per, 75):.0f})")
    return out, per_rep


# revision 5
# speedup vs baseline: 1.0689x; 1.0689x over previous
"""ResNet bottleneck block (dense_cnn) on 8 Trainium2 NeuronCores.

Reference computation (NCHW, fp32):
    t1  = relu(s1 * conv1x1(x, w1, stride=2) + b1)     # 512 -> 256, 28x28 -> 14x14
    t2  = relu(s2 * conv3x3(t1, w2, pad=1)   + b2)     # 256 -> 256
    t3  =      s3 * conv1x1(t2, w3)          + b3      # 256 -> 1024
    idn =      s4 * conv1x1(x, w4, stride=2) + b4      # 512 -> 1024
    out = relu(t3 + idn)                               # (64, 1024, 14, 14)

Strategy:
  - Data-parallel over batch: 64 images -> 8 cores x 8 images.
  - Host-side prep (numpy, cheap): subsample x to its even (h, w)
    positions, fold BN scales into conv weights, transpose weights to
    [ci, co], cast to bf16 (rel err ~5e-3 << the 2e-2 gate at ~2x the
    PE/DMA throughput of fp32 paths).
  - On-chip: every conv is a matmul with channels on partitions and
    (image, h, w) on the free dim (392 columns = 2 images).  The 3x3
    conv is 9 shifted matmuls accumulating in PSUM over zero-padded
    16-wide planes of t1.  Weight-stationary ordering amortizes
    LDWEIGHTS (hidden by the PE reorder window).
  - PSUM: each half-stage owns a [128, 2048]-f32 tile (4 banks); the 4
    image groups live at 512-col bank-aligned offsets so a whole
    m-group drains with ONE wide ACT/DVE instruction (bias+relu fused)
    instead of 4-16 narrow ones; engine fixed overhead (~0.2us/instr)
    amortizes 4-8x.
  - Stage 1 is k-outer (both conv1 output halves accumulate in PSUM
    across arriving xs chunks) so each 0.4MB xs chunk unlocks 8
    matmuls and the input stream stays ahead of the PE.
  - Stage 3 issues the conv4 (residual, xs-only) matmuls before conv3
    so the PE fills while stage 2 still drains; conv3 accumulates into
    the same PSUM so the add + final relu are one pass.
  - DMA queues: SP-HWDGE carries w1+xs(+biases) -- a pure-dma_start
    instruction stream, so the next For_i iteration's loads prefetch as
    soon as buffers free instead of queueing behind drain work.
    ACT-HWDGE carries w2/w3/w4 (not needed until ~10us in) and the last
    two output chunks; the Pool-engine SWDGE streams the rest of the
    output.  t1pad zeroing only touches the halo border (~900 cols vs
    4352 for full planes).
  - Post-finalize IR passes: drop InstLdweights duplicated by
    legalization for repeated stationary operands, thin matmul
    semaphore updates to the waited counts, and strip semaphore waits
    already implied by earlier waits on the same engine.

Measured per-execution device time via a hardware For_i loop slope:
~66us on the axon-tunneled trn2 (baseline of this session: ~80us).
Pure-PE floor for 368 matmuls of 392 cols at the measured back-to-back
rate (24ns + 0.52ns/col under full-power P0 clocks) is ~64us.
"""

import os

import numpy as np

import concourse.mybir as mybir
import concourse.tile as tile
from concourse import bacc
from concourse.bass_utils import run_bass_kernel_spmd

F32 = mybir.dt.float32
BF16 = mybir.dt.bfloat16
F32R = mybir.dt.float32r
I32 = mybir.dt.int32

N_CORES = 8
B = 8              # images per core
HW = 14            # output spatial
P = HW * HW        # 196 per image plane (compact)
PB = B * P         # 1568
WP = 16            # padded row width for the 3x3 conv input
Q = HW * WP        # 224 (padded-plane columns per image in conv2 psum)
PADQ = 17 * WP     # 272 per-image padded plane (1 extra slack row)
NG = 2             # images per matmul group
G = B // NG        # 4 groups
NF = NG * P        # 392: compact moving-operand free size
GP = 512           # psum columns per image group (bank-aligned)

# Compute dtype for matmuls: "f32r" (fp32 storage, TF32-like multiply,
# full PE rate), "f32" (exact, 1/4 rate), "bf16".
COMPUTE_DT = os.environ.get("BOT_DT", "bf16")
# Debug: build only the first N stages (1..3) for per-stage HW timing.
STAGES = int(os.environ.get("BOT_STAGES", "3"))

_CACHE = {}


def _dedupe_ldweights(nc):
    """Drop InstLdweights identical to the previous one in the PE stream.

    Legalization emits one weight load per matmul; when consecutive
    matmuls share the stationary operand the repeated ~107ns loads are
    pure PE overhead.  Only waits/updates-free duplicates are dropped,
    so semaphore counts are unchanged.
    """

    def ap_sig(ap):
        try:
            ml = ap.memorylocation
            name = ml.name if ml is not None else None
        except Exception:
            name = None
        off = getattr(ap, "offset", None)
        try:
            dims = tuple((d.num_elem, d.step) for d in ap.aps)
        except Exception:
            dims = str(ap)
        return (name, off, dims)

    removed = 0
    for fn in nc.m.functions:
        for blk in fn.blocks:
            insts = blk.instructions
            last_sig = None
            keep = []
            for inst in insts:
                if isinstance(inst, mybir.InstLdweights):
                    sig = (
                        ap_sig(inst.ins[0]),
                        getattr(inst, "perf_mode", None),
                        getattr(inst, "is_transpose", None),
                        getattr(inst, "tile_position", None),
                    )
                    si = inst.sync_info
                    clean = si is None or (
                        len(si.on_wait) == 0 and len(si.on_update) == 0
                    )
                    if clean and sig == last_sig:
                        removed += 1
                        continue
                    last_sig = sig
                elif not isinstance(inst, mybir.InstMatmult):
                    if isinstance(
                        inst,
                        (mybir.InstUnconditionalBranch, mybir.InstCall),
                    ):
                        last_sig = None
                keep.append(inst)
            if len(keep) != len(insts):
                del insts[:]
                insts.extend(keep)
    return removed


def _strip_redundant_waits(nc):
    """Remove semaphore waits already implied by earlier waits.

    Engines execute their instruction stream in order, so once an
    instruction on engine E has waited for sem >= v, every later wait on
    E for sem >= v' with v' <= v is a no-op.  Each retired wait still
    costs the sequencer dispatch time, so stripping them shortens the
    per-instruction issue path.  Tracking is per block and resets at
    event-semaphore (barrier/reset) instructions, which is conservative
    for loop back-edges.
    """
    removed = 0
    for fn in nc.m.functions:
        for blk in fn.blocks:
            seen = {}  # (engine, sem_id) -> max value waited
            for inst in blk.instructions:
                if isinstance(inst, mybir.InstEventSemaphore):
                    seen = {k: v for k, v in seen.items() if k[0] != inst.engine}
                    # barriers also imply cross-engine sync; be safe:
                    seen = {}
                    continue
                si = inst.sync_info
                if si is None or not si.on_wait:
                    continue
                kept = []
                for w in si.on_wait:
                    if (
                        getattr(w, "sync_type", None) == "semaphore"
                        and getattr(w, "wait_mode", None) == "sem-ge-imm"
                        and getattr(w, "wait_value", None) is not None
                    ):
                        key = (inst.engine, w.id)
                        if seen.get(key, -1) >= w.wait_value:
                            removed += 1
                            continue
                        seen[key] = w.wait_value
                    kept.append(w)
                if len(kept) != len(si.on_wait):
                    si.on_wait = kept
    return removed


def _thin_pe_sem_updates(nc):
    """Drop matmul semaphore increments no consumer distinguishes.

    Every matmul increments the PE progress semaphore (~26ns serialized
    EVT_SEM write each), but consumers wait on only a few distinct
    counts.  Keep exactly the increments at waited cumulative counts and
    renumber every wait to its rank among kept values.  Applied only to
    semaphores whose updates are exclusively matmul sem-inc(+1) and
    whose waits are all sem-ge-imm, so semantics are preserved.
    """
    removed = 0
    for fn in nc.m.functions:
        upd, bad, waits = {}, set(), {}
        for blk in fn.blocks:
            for inst in blk.instructions:
                si = inst.sync_info
                if not si:
                    continue
                for u in si.on_update:
                    if getattr(u, "sync_type", None) != "semaphore":
                        continue
                    if (
                        getattr(u, "update_mode", None) != "sem-inc"
                        or getattr(u, "update_value", None) != 1
                        or not isinstance(inst, mybir.InstMatmult)
                    ):
                        bad.add(u.id)
                    upd.setdefault(u.id, []).append((inst, u))
                for w in si.on_wait:
                    if getattr(w, "sync_type", None) != "semaphore":
                        continue
                    if (
                        getattr(w, "wait_mode", None) != "sem-ge-imm"
                        or getattr(w, "wait_value", None) is None
                    ):
                        bad.add(getattr(w, "id", None))
                        continue
                    waits.setdefault(w.id, []).append(w)
        for sid, entries in upd.items():
            if sid in bad or len(entries) < 8:
                continue
            vals = sorted({w.wait_value for w in waits.get(sid, [])})
            pos = [v for v in vals if v >= 1]
            if not pos or pos[-1] > len(entries):
                continue
            keep = set(pos)
            rank = {v: i + 1 for i, v in enumerate(pos)}
            for idx, (inst, u) in enumerate(entries):
                if idx + 1 not in keep:
                    inst.sync_info.on_update = [
                        x for x in inst.sync_info.on_update if x is not u
                    ]
                    removed += 1
            for w in waits.get(sid, []):
                if w.wait_value >= 1:
                    w.wait_value = rank[w.wait_value]
    return removed


def _build_nc(reps=1, loop_n=0):
    """loop_n > 0 wraps the body in a hardware For_i loop (timing only)."""
    act_dt = {"bf16": BF16, "f32": F32, "f32r": F32R}[COMPUTE_DT]
    # bf16 output halves the out DMA and the drain write traffic; host
    # upcasts after gather.  Error stays ~1e-3 << the 2e-2 gate.
    out_dt = BF16 if (act_dt == BF16
                      and os.environ.get("BOT_OUT_BF16", "1") == "1") else F32

    nc = bacc.Bacc()
    xs_d = nc.declare_dram_parameter("xs", [512, PB], act_dt, isOutput=False)
    w1_d = nc.declare_dram_parameter("w1t", [512, 256], act_dt, isOutput=False)
    w2_d = nc.declare_dram_parameter("w2t", [9 * 256, 256], act_dt, isOutput=False)
    w3_d = nc.declare_dram_parameter("w3t", [256, 1024], act_dt, isOutput=False)
    w4_d = nc.declare_dram_parameter("w4t", [512, 1024], act_dt, isOutput=False)
    b1_d = nc.declare_dram_parameter("b1p", [128, 2], F32, isOutput=False)
    b2_d = nc.declare_dram_parameter("b2p", [128, 2], F32, isOutput=False)
    b34_d = nc.declare_dram_parameter("b34p", [128, 8], F32, isOutput=False)
    out_d = nc.declare_dram_parameter("out", [1024, PB], out_dt, isOutput=True)

    relu = mybir.ActivationFunctionType.Relu
    alu_add = mybir.AluOpType.add
    alu_max = mybir.AluOpType.max

    # stage-1/2 m-halves whose drain runs on ACT (rest on DVE)
    s12_act = {
        int(x)
        for x in os.environ.get("BOT_S12_ACT", "0").split(",")
        if x != ""
    }
    # stage-3 m-groups whose drain runs on ACT (rest on DVE)
    s3_act = {
        int(x)
        for x in os.environ.get("BOT_S3_ACT", "1,5").split(",")
        if x != ""
    }
    # stage-3 m-groups whose output DMA rides the ACT HWDGE queue
    out_act = {
        int(x)
        for x in os.environ.get("BOT_OUT_ACT", "6,7").split(",")
        if x != ""
    }

    def post(on_act, dst, src, bias_ap):
        # relu(src + bias) -> dst on the chosen engine
        if on_act:
            nc.scalar.activation(dst, src, relu, bias=bias_ap)
        else:
            nc.vector.tensor_scalar(dst, src, bias_ap, 0.0, alu_add, alu_max)

    import contextlib

    with tile.TileContext(nc) as tc:
        with (
            tc.tile_pool(name="consts", bufs=2) as consts,
            tc.tile_pool(name="psum", bufs=8, space="PSUM") as psum,
            tc.tile_pool(name="outp", bufs=3) as outp,
            (
                tc.For_i(0, loop_n, 1, hint_engines=(mybir.EngineType.PE,),
                         staggered_reset=True)
                if loop_n
                else contextlib.nullcontext()
            ),
        ):
            for _rep in range(reps):
                # --- SP queue: w1 first (unblocks the PE), then the xs
                # chunks in consumption order, then the small biases.
                # A pure-dma_start stream: next iteration's loads issue
                # as soon as pool buffers free.
                w1_t = consts.tile([128, 4 * 256], act_dt, tag="w1")
                nc.sync.dma_start(
                    out=w1_t.rearrange("p (k c) -> p k c", k=4),
                    in_=w1_d.rearrange("(k p) c -> p k c", p=128),
                )
                w1_sb = [w1_t[:, k * 256:(k + 1) * 256] for k in range(4)]
                b1_sb = consts.tile([128, 2], F32, tag="b1")
                nc.sync.dma_start(out=b1_sb, in_=b1_d[:, :])
                xs_sb = [
                    consts.tile([128, PB], act_dt, tag=f"xs_{k}", name=f"xs_{k}")
                    for k in range(4)
                ]
                for k in range(4):
                    nc.sync.dma_start(
                        out=xs_sb[k], in_=xs_d[k * 128:(k + 1) * 128, :]
                    )
                b2_sb = consts.tile([128, 2], F32, tag="b2")
                nc.sync.dma_start(out=b2_sb, in_=b2_d[:, :])
                b34_sb = consts.tile([128, 8], F32, tag="b34")
                nc.sync.dma_start(out=b34_sb, in_=b34_d[:, :])

                # --- ACT queue: w2/w3/w4, needed from ~10us in ---
                w2_t = consts.tile([128, 18 * 256], act_dt, tag="w2")
                nc.scalar.dma_start(
                    out=w2_t.rearrange("p (c n) -> p c n", c=18),
                    in_=w2_d.rearrange("(c p) n -> p c n", p=128),
                )
                w2_sb = [
                    [
                        w2_t[:, (tap * 2 + k) * 256:(tap * 2 + k + 1) * 256]
                        for k in range(2)
                    ]
                    for tap in range(9)
                ]
                w3_t = consts.tile([128, 2 * 1024], act_dt, tag="w3")
                nc.scalar.dma_start(
                    out=w3_t.rearrange("p (k c) -> p k c", k=2),
                    in_=w3_d.rearrange("(k p) c -> p k c", p=128),
                )
                w3_sb = [w3_t[:, k * 1024:(k + 1) * 1024] for k in range(2)]
                w4_t = consts.tile([128, 4 * 1024], act_dt, tag="w4")
                nc.scalar.dma_start(
                    out=w4_t.rearrange("p (k c) -> p k c", k=4),
                    in_=w4_d.rearrange("(k p) c -> p k c", p=128),
                )
                w4_sb = [w4_t[:, k * 1024:(k + 1) * 1024] for k in range(4)]

                # --- t1 padded planes: zero only the halo border ---
                t1pad = []
                for k in range(2):
                    t = consts.tile([128, B * PADQ], act_dt, tag=f"t1p_{k}")
                    if os.environ.get("BOT_MEMSET", "border") == "full":
                        nc.vector.memset(t, 0.0)
                    else:
                        pl = t.rearrange("p (i q) -> p i q", i=B)
                        # rows 15..16 of every image
                        nc.vector.memset(pl[:, :, 15 * WP:], 0.0)
                        # row 0 of every image
                        nc.vector.memset(pl[:, :, 0:WP], 0.0)
                        # cols 15,0-of-next-row for rows 1..14
                        nc.vector.memset(
                            pl.rearrange("p i (h w) -> p i h w", w=WP)[
                                :, :, 1:15, 15:16
                            ],
                            0.0,
                        )
                        nc.vector.memset(
                            pl.rearrange("p i (h w) -> p i h w", w=WP)[
                                :, :, 1:15, 0:1
                            ],
                            0.0,
                        )
                    t1pad.append(t)
                t2_sb = []
                for k in range(2):
                    t = consts.tile([128, PB], act_dt, tag=f"t2_{k}")
                    t2_sb.append(t)

                # --- stage 1: conv1 + relu, scattered into padded planes.
                # k-outer: each arriving xs chunk feeds 8 matmuls (both
                # output halves), PSUM accumulates across chunks.
                for m in range(2 if STAGES >= 1 else 0):
                    pss = [psum.tile([128, NF], F32, tag="ps", name=f"ps{_g}")
                           for _g in range(G)]
                    for k in range(4):
                        for g in range(G):
                            nc.tensor.matmul(
                                pss[g][:, :],
                                w1_sb[k][:, m * 128:(m + 1) * 128],
                                xs_sb[k][:, g * NF:(g + 1) * NF],
                                start=(k == 0),
                                stop=(k == 3),
                            )
                    for g in range(G):
                        dst = t1pad[m][
                            :, g * NG * PADQ:(g * NG + NG) * PADQ
                        ].rearrange("p (n h w) -> p n h w", h=17, w=WP)[
                            :, :, 1:15, 1:15
                        ]
                        post((m * 4 + g) % 2 == 0, dst, pss[g][:, :],
                             b1_sb[:, m:m + 1])

                # --- stage 2: conv2 (3x3 as 9 shifted matmuls) + relu ---
                # weight-stationary: each tap/k weight feeds all 4 image
                # groups; moving operand is a 4-level shifted view of the
                # padded planes so pad columns are never streamed
                for m in range(2 if STAGES >= 2 else 0):
                    pss = [psum.tile([128, NF], F32, tag="ps", name=f"ps{_g}")
                           for _g in range(G)]
                    i = 0
                    for tap in range(9):
                        dy, dx = divmod(tap, 3)
                        for k in range(2):
                            for g in range(G):
                                seg = t1pad[k][
                                    :, g * NG * PADQ:(g * NG + NG) * PADQ
                                ].rearrange(
                                    "p (n h w) -> p n h w", h=17, w=WP
                                )[:, :, dy:dy + HW, dx:dx + HW]
                                nc.tensor.matmul(
                                    pss[g][:, :],
                                    w2_sb[tap][k][:, m * 128:(m + 1) * 128],
                                    seg,
                                    start=(i == 0),
                                    stop=(i == 17),
                                )
                            i += 1
                    for g in range(G):
                        post((m * 4 + g) % 2 == 0,
                             t2_sb[m][:, g * NF:(g + 1) * NF],
                             pss[g][:, :], b2_sb[:, m:m + 1])

                # --- stage 3: conv3 + residual conv4 in one PSUM, relu ---
                for m in range(8 if STAGES >= 3 else 0):
                    pss = [psum.tile([128, NF], F32, tag="ps", name=f"ps{_g}")
                           for _g in range(G)]
                    # conv4 first: it depends only on xs, so its matmuls can
                    # fill the PE while stage 2 still drains
                    for k in range(4):
                        for g in range(G):
                            nc.tensor.matmul(
                                pss[g][:, :],
                                w4_sb[k][:, m * 128:(m + 1) * 128],
                                xs_sb[k][:, g * NF:(g + 1) * NF],
                                start=(k == 0),
                                stop=False,
                            )
                    for k in range(2):
                        for g in range(G):
                            nc.tensor.matmul(
                                pss[g][:, :],
                                w3_sb[k][:, m * 128:(m + 1) * 128],
                                t2_sb[k][:, g * NF:(g + 1) * NF],
                                start=False,
                                stop=(k == 1),
                            )
                    ot = outp.tile([128, PB], out_dt, tag="ot")
                    for g in range(G):
                        post(g == 3, ot[:, g * NF:(g + 1) * NF],
                             pss[g][:, :], b34_sb[:, m:m + 1])
                    if m in out_act:
                        nc.scalar.dma_start(
                            out=out_d[m * 128:(m + 1) * 128, :], in_=ot,
                        )
                    else:
                        nc.gpsimd.dma_start(
                            out=out_d[m * 128:(m + 1) * 128, :], in_=ot,
                        )
                if STAGES < 3:
                    ot = outp.tile([128, PB], out_dt, tag="ot")
                    src_t = (t2_sb[0] if STAGES >= 2 else
                             (t1pad[0][:, 0:PB] if STAGES >= 1
                              else xs_sb[0][:, 0:PB]))
                    nc.vector.tensor_copy(ot, src_t)
                    nc.gpsimd.dma_start(out=out_d[0:128, :], in_=ot)
    nc.finalize()
    if os.environ.get("BOT_LDW_DEDUP", "1") == "1":
        n = _dedupe_ldweights(nc)
        if os.environ.get("BOT_VERBOSE"):
            print(f"[kernel] deduped {n} InstLdweights")
    if os.environ.get("BOT_SEM_THIN", "1") == "1":
        n = _thin_pe_sem_updates(nc)
        if os.environ.get("BOT_VERBOSE"):
            print(f"[kernel] thinned {n} matmul sem updates")
    if os.environ.get("BOT_WAIT_STRIP", "1") == "1":
        n = _strip_redundant_waits(nc)
        if os.environ.get("BOT_VERBOSE"):
            print(f"[kernel] stripped {n} redundant waits")
    return nc


def _prep(x, w1, w2, w3, w4, s1, b1, s2, b2, s3, b3, s4, b4):
    """Host-side input prep: shard, fold BN, transpose. All numpy."""
    if COMPUTE_DT == "bf16":
        import ml_dtypes

        cdt = np.dtype(ml_dtypes.bfloat16)
    else:
        cdt = np.dtype(np.float32)

    # x -> even positions, (core, c, n, h*14+w) channel-major partition lines
    xs = x[:, :, ::2, ::2].reshape(N_CORES, B, 512, P).transpose(0, 2, 1, 3)
    xs = np.ascontiguousarray(xs).reshape(N_CORES, 512, PB).astype(cdt)

    w1f = (w1[:, :, 0, 0] * s1[:, None]).T                    # (512, 256)
    w2f = w2 * s2[:, None, None, None]                        # (256,256,3,3)
    w2t = np.stack(
        [w2f[:, :, dy, dx].T for dy in range(3) for dx in range(3)]
    ).reshape(9 * 256, 256)                                   # (2304, 256)
    w3f = (w3[:, :, 0, 0] * s3[:, None]).T                    # (256, 1024)
    w4f = (w4[:, :, 0, 0] * s4[:, None]).T                    # (512, 1024)

    com = {
        "w1t": np.ascontiguousarray(w1f).astype(cdt),
        "w2t": np.ascontiguousarray(w2t).astype(cdt),
        "w3t": np.ascontiguousarray(w3f).astype(cdt),
        "w4t": np.ascontiguousarray(w4f).astype(cdt),
        "b1p": np.ascontiguousarray(b1.reshape(2, 128).T).astype(np.float32),
        "b2p": np.ascontiguousarray(b2.reshape(2, 128).T).astype(np.float32),
        "b34p": np.ascontiguousarray(
            (b3 + b4).reshape(8, 128).T
        ).astype(np.float32),
    }
    return [{"xs": xs[c], **com} for c in range(N_CORES)]


def _gather(results):
    out = np.empty((64, 1024, HW, HW), np.float32)
    for c, r in enumerate(results):
        o = r["out"].astype(np.float32).reshape(1024, B, HW, HW)
        out[c * B:(c + 1) * B] = o.transpose(1, 0, 2, 3)
    return out


def _get_nc(reps=1, loop_n=0):
    key = ("nc", reps, loop_n)
    if key not in _CACHE:
        _CACHE[key] = _build_nc(reps, loop_n)
    return _CACHE[key]


def _run(in_maps, **kwargs):
    return run_bass_kernel_spmd(
        _get_nc(), in_maps, list(range(N_CORES)), **kwargs
    )


def kernel(**inputs):
    in_maps = _prep(**inputs)
    res = _run(in_maps)
    return _gather(res.results)


def _pjrt_runner(nc, in_maps):
    """Compile nc once; return (run_once, run_batch, results).

    run_once(): one blocking execution. run_batch(n): n pipelined
    executions, blocking at the end; returns elapsed seconds. results:
    first run's outputs as a list of per-core dicts.
    """
    import time

    import jax
    import numpy as np_
    from jax.sharding import Mesh, NamedSharding, PartitionSpec
    from jax.experimental.shard_map import shard_map

    from concourse import bass2jax, mybir as mb

    bass2jax.install_neuronx_cc_hook()
    part_name = nc.partition_id_tensor.name if nc.partition_id_tensor else None
    in_names, out_names, out_avals = [], [], []
    for alloc in nc.m.functions[0].allocations:
        if not isinstance(alloc, mb.MemoryLocationSet):
            continue
        name = alloc.memorylocations[0].name
        if alloc.kind == "ExternalInput":
            if name != part_name:
                in_names.append(name)
        elif alloc.kind == "ExternalOutput":
            out_names.append(name)
            out_avals.append(
                jax.core.ShapedArray(
                    tuple(alloc.tensor_shape), mb.dt.np(alloc.dtype)
                )
            )
    all_names = in_names + out_names + ([part_name] if part_name else [])

    def _body(*args):
        operands = list(args)
        if part_name is not None:
            operands.append(bass2jax.partition_id_tensor())
        outs = bass2jax._bass_exec_p.bind(
            *operands,
            out_avals=tuple(out_avals),
            in_names=tuple(all_names),
            out_names=tuple(out_names),
            lowering_input_output_aliases=(),
            sim_require_finite=False,
            sim_require_nnan=False,
            nc=nc,
        )
        return tuple(outs)

    devices = jax.devices()[:N_CORES]
    mesh = Mesh(np_.asarray(devices), ("core",))
    nspec = len(in_names) + len(out_names)
    sharded = jax.jit(
        shard_map(
            _body,
            mesh=mesh,
            in_specs=(PartitionSpec("core"),) * nspec,
            out_specs=(PartitionSpec("core"),) * len(out_names),
            check_rep=False,
        ),
        keep_unused=True,
    )

    sh = NamedSharding(mesh, PartitionSpec("core"))
    dev_args = [
        jax.device_put(
            np_.concatenate([in_maps[c][n] for c in range(N_CORES)], axis=0), sh
        )
        for n in in_names
    ] + [
        jax.device_put(
            np_.zeros((N_CORES * a.shape[0], *a.shape[1:]), a.dtype), sh
        )
        for a in out_avals
    ]

    outs = jax.block_until_ready(sharded(*dev_args))  # compile + warm

    results = [
        {
            n: np_.asarray(outs[i]).reshape(N_CORES, *out_avals[i].shape)[c]
            for i, n in enumerate(out_names)
        }
        for c in range(N_CORES)
    ]

    def run_once():
        jax.block_until_ready(sharded(*dev_args))

    def run_batch(n):
        t0 = time.monotonic()
        r = None
        for _ in range(n):
            r = sharded(*dev_args)
        jax.block_until_ready(r)
        return time.monotonic() - t0

    return run_once, run_batch, results


def kernel_timed(**inputs):
    """Run + estimate steady-state per-execution device time (ns).

    NTFF profiling is unavailable under this axon client and per-call
    wall time is ~80ms of tunnel overhead, so device time is measured
    with a hardware For_i loop: two NEFFs run the 2-rep body loop_n1 /
    loop_n2 times; the wall-time slope over (loop_n2-loop_n1)*2 reps
    cancels the per-call overhead.  Returns (out, exec_time_ns).
    """
    import time

    import numpy as np_

    n1 = int(os.environ.get("BOT_LOOP_N1", "32"))
    n2 = int(os.environ.get("BOT_LOOP_N2", "224"))
    trials = int(os.environ.get("BOT_LOOP_TRIALS", "30"))
    in_maps = _prep(**inputs)

    res = _run(in_maps)
    out = _gather(res.results)

    once1, _, _ = _pjrt_runner(_get_nc(2, loop_n=n1), in_maps)
    once2, _, _ = _pjrt_runner(_get_nc(2, loop_n=n2), in_maps)

    def s(f):
        t0 = time.monotonic()
        f()
        return time.monotonic() - t0

    for _ in range(2):
        s(once1), s(once2)
    t1s, t2s = [], []
    for _ in range(trials):
        t1s.append(s(once1))
        t2s.append(s(once2))
    t1s, t2s = np_.array(t1s), np_.array(t2s)
    per = (t2s - t1s) / (n2 - n1) / 2 * 1e9
    per_rep = int(np_.median(per))
    print(f"[bench] For_i loop slope over {trials} trials: {per_rep} ns "
          f"(iqr {np_.percentile(per, 25):.0f}-{np_.percentile(per, 75):.0f})")
    return out, per_rep


# revision 6
# speedup vs baseline: 1.1214x; 1.0492x over previous
"""ResNet bottleneck block (dense_cnn) on 8 Trainium2 NeuronCores.

Reference computation (NCHW, fp32):
    t1  = relu(s1 * conv1x1(x, w1, stride=2) + b1)     # 512 -> 256, 28x28 -> 14x14
    t2  = relu(s2 * conv3x3(t1, w2, pad=1)   + b2)     # 256 -> 256
    t3  =      s3 * conv1x1(t2, w3)          + b3      # 256 -> 1024
    idn =      s4 * conv1x1(x, w4, stride=2) + b4      # 512 -> 1024
    out = relu(t3 + idn)                               # (64, 1024, 14, 14)

Strategy:
  - Data-parallel over batch: 64 images -> 8 cores x 8 images.
  - Host-side prep (numpy, cheap): subsample x to its even (h, w)
    positions, fold BN scales into conv weights, transpose weights to
    [ci, co], cast to bf16 (rel err ~5e-3 << the 2e-2 gate at ~2x the
    PE/DMA throughput of fp32 paths).
  - On-chip: every conv is a matmul with channels on partitions and
    (image, h, w) on the free dim (392 columns = 2 images).  The 3x3
    conv is 9 shifted matmuls accumulating in PSUM over zero-padded
    16-wide planes of t1.  Weight-stationary ordering amortizes
    LDWEIGHTS (hidden by the PE reorder window).
  - PSUM: each half-stage owns a [128, 2048]-f32 tile (4 banks); the 4
    image groups live at 512-col bank-aligned offsets so a whole
    m-group drains with ONE wide ACT/DVE instruction (bias+relu fused)
    instead of 4-16 narrow ones; engine fixed overhead (~0.2us/instr)
    amortizes 4-8x.
  - Stage 1 is k-outer (both conv1 output halves accumulate in PSUM
    across arriving xs chunks) so each 0.4MB xs chunk unlocks 8
    matmuls and the input stream stays ahead of the PE.
  - Stage 3 issues the conv4 (residual, xs-only) matmuls before conv3
    so the PE fills while stage 2 still drains; conv3 accumulates into
    the same PSUM so the add + final relu are one pass.
  - DMA queues: SP-HWDGE carries w1+xs(+biases) -- a pure-dma_start
    instruction stream, so the next For_i iteration's loads prefetch as
    soon as buffers free instead of queueing behind drain work.
    ACT-HWDGE carries w2/w3/w4 (not needed until ~10us in) and the last
    two output chunks; the Pool-engine SWDGE streams the rest of the
    output.  t1pad zeroing only touches the halo border (~900 cols vs
    4352 for full planes).
  - Post-finalize IR passes: drop InstLdweights duplicated by
    legalization for repeated stationary operands, thin matmul
    semaphore updates to the waited counts, and strip semaphore waits
    already implied by earlier waits on the same engine.

Measured per-execution device time via a hardware For_i loop slope:
~66us on the axon-tunneled trn2 (baseline of this session: ~80us).
Pure-PE floor for 368 matmuls of 392 cols at the measured back-to-back
rate (24ns + 0.52ns/col under full-power P0 clocks) is ~64us.
"""

import os

import numpy as np

import concourse.mybir as mybir
import concourse.tile as tile
from concourse import bacc
from concourse.bass_utils import run_bass_kernel_spmd

F32 = mybir.dt.float32
BF16 = mybir.dt.bfloat16
F32R = mybir.dt.float32r
I32 = mybir.dt.int32

N_CORES = 8
B = 8              # images per core
HW = 14            # output spatial
P = HW * HW        # 196 per image plane (compact)
PB = B * P         # 1568
WP = 16            # padded row width for the 3x3 conv input
Q = HW * WP        # 224 (padded-plane columns per image in conv2 psum)
PADQ = 17 * WP     # 272 per-image padded plane (1 extra slack row)
NG = 2             # images per matmul group
G = B // NG        # 4 groups
NF = NG * P        # 392: compact moving-operand free size
GP = 512           # psum columns per image group (bank-aligned)

# Compute dtype for matmuls: "f32r" (fp32 storage, TF32-like multiply,
# full PE rate), "f32" (exact, 1/4 rate), "bf16".
COMPUTE_DT = os.environ.get("BOT_DT", "bf16")
# Debug: build only the first N stages (1..3) for per-stage HW timing.
STAGES = int(os.environ.get("BOT_STAGES", "3"))

_CACHE = {}


def _dedupe_ldweights(nc):
    """Drop InstLdweights identical to the previous one in the PE stream.

    Legalization emits one weight load per matmul; when consecutive
    matmuls share the stationary operand the repeated ~107ns loads are
    pure PE overhead.  Only waits/updates-free duplicates are dropped,
    so semaphore counts are unchanged.
    """

    def ap_sig(ap):
        try:
            ml = ap.memorylocation
            name = ml.name if ml is not None else None
        except Exception:
            name = None
        off = getattr(ap, "offset", None)
        try:
            dims = tuple((d.num_elem, d.step) for d in ap.aps)
        except Exception:
            dims = str(ap)
        return (name, off, dims)

    removed = 0
    for fn in nc.m.functions:
        for blk in fn.blocks:
            insts = blk.instructions
            last_sig = None
            keep = []
            for inst in insts:
                if isinstance(inst, mybir.InstLdweights):
                    sig = (
                        ap_sig(inst.ins[0]),
                        getattr(inst, "perf_mode", None),
                        getattr(inst, "is_transpose", None),
                        getattr(inst, "tile_position", None),
                    )
                    si = inst.sync_info
                    clean = si is None or (
                        len(si.on_wait) == 0 and len(si.on_update) == 0
                    )
                    if clean and sig == last_sig:
                        removed += 1
                        continue
                    last_sig = sig
                elif not isinstance(inst, mybir.InstMatmult):
                    if isinstance(
                        inst,
                        (mybir.InstUnconditionalBranch, mybir.InstCall),
                    ):
                        last_sig = None
                keep.append(inst)
            if len(keep) != len(insts):
                del insts[:]
                insts.extend(keep)
    return removed


def _strip_redundant_waits(nc):
    """Remove semaphore waits already implied by earlier waits.

    Engines execute their instruction stream in order, so once an
    instruction on engine E has waited for sem >= v, every later wait on
    E for sem >= v' with v' <= v is a no-op.  Each retired wait still
    costs the sequencer dispatch time, so stripping them shortens the
    per-instruction issue path.  Tracking is per block and resets at
    event-semaphore (barrier/reset) instructions, which is conservative
    for loop back-edges.
    """
    removed = 0
    for fn in nc.m.functions:
        for blk in fn.blocks:
            seen = {}  # (engine, sem_id) -> max value waited
            for inst in blk.instructions:
                if isinstance(inst, mybir.InstEventSemaphore):
                    seen = {k: v for k, v in seen.items() if k[0] != inst.engine}
                    # barriers also imply cross-engine sync; be safe:
                    seen = {}
                    continue
                si = inst.sync_info
                if si is None or not si.on_wait:
                    continue
                kept = []
                for w in si.on_wait:
                    if (
                        getattr(w, "sync_type", None) == "semaphore"
                        and getattr(w, "wait_mode", None) == "sem-ge-imm"
                        and getattr(w, "wait_value", None) is not None
                    ):
                        key = (inst.engine, w.id)
                        if seen.get(key, -1) >= w.wait_value:
                            removed += 1
                            continue
                        seen[key] = w.wait_value
                    kept.append(w)
                if len(kept) != len(si.on_wait):
                    si.on_wait = kept
    return removed


def _thin_pe_sem_updates(nc):
    """Drop matmul semaphore increments no consumer distinguishes.

    Every matmul increments the PE progress semaphore (~26ns serialized
    EVT_SEM write each), but consumers wait on only a few distinct
    counts.  Keep exactly the increments at waited cumulative counts and
    renumber every wait to its rank among kept values.  Applied only to
    semaphores whose updates are exclusively matmul sem-inc(+1) and
    whose waits are all sem-ge-imm, so semantics are preserved.
    """
    removed = 0
    for fn in nc.m.functions:
        upd, bad, waits = {}, set(), {}
        for blk in fn.blocks:
            for inst in blk.instructions:
                si = inst.sync_info
                if not si:
                    continue
                for u in si.on_update:
                    if getattr(u, "sync_type", None) != "semaphore":
                        continue
                    if (
                        getattr(u, "update_mode", None) != "sem-inc"
                        or getattr(u, "update_value", None) != 1
                        or not isinstance(inst, mybir.InstMatmult)
                    ):
                        bad.add(u.id)
                    upd.setdefault(u.id, []).append((inst, u))
                for w in si.on_wait:
                    if getattr(w, "sync_type", None) != "semaphore":
                        continue
                    if (
                        getattr(w, "wait_mode", None) != "sem-ge-imm"
                        or getattr(w, "wait_value", None) is None
                    ):
                        bad.add(getattr(w, "id", None))
                        continue
                    waits.setdefault(w.id, []).append(w)
        for sid, entries in upd.items():
            if sid in bad or len(entries) < 8:
                continue
            vals = sorted({w.wait_value for w in waits.get(sid, [])})
            pos = [v for v in vals if v >= 1]
            if not pos or pos[-1] > len(entries):
                continue
            keep = set(pos)
            rank = {v: i + 1 for i, v in enumerate(pos)}
            for idx, (inst, u) in enumerate(entries):
                if idx + 1 not in keep:
                    inst.sync_info.on_update = [
                        x for x in inst.sync_info.on_update if x is not u
                    ]
                    removed += 1
            for w in waits.get(sid, []):
                if w.wait_value >= 1:
                    w.wait_value = rank[w.wait_value]
    return removed


def _build_nc(reps=1, loop_n=0):
    """loop_n > 0 wraps the body in a hardware For_i loop (timing only)."""
    act_dt = {"bf16": BF16, "f32": F32, "f32r": F32R}[COMPUTE_DT]
    # bf16 output halves the out DMA and the drain write traffic; host
    # upcasts after gather.  Error stays ~1e-3 << the 2e-2 gate.
    out_dt = BF16 if (act_dt == BF16
                      and os.environ.get("BOT_OUT_BF16", "1") == "1") else F32

    nc = bacc.Bacc()
    xs_d = nc.declare_dram_parameter("xs", [512, PB], act_dt, isOutput=False)
    w1_d = nc.declare_dram_parameter("w1t", [512, 256], act_dt, isOutput=False)
    w2_d = nc.declare_dram_parameter("w2t", [9 * 256, 256], act_dt, isOutput=False)
    w3_d = nc.declare_dram_parameter("w3t", [256, 1024], act_dt, isOutput=False)
    w4_d = nc.declare_dram_parameter("w4t", [512, 1024], act_dt, isOutput=False)
    b1_d = nc.declare_dram_parameter("b1p", [128, 2], F32, isOutput=False)
    b2_d = nc.declare_dram_parameter("b2p", [128, 2], F32, isOutput=False)
    b34_d = nc.declare_dram_parameter("b34p", [128, 8], F32, isOutput=False)
    out_d = nc.declare_dram_parameter("out", [1024, PB], out_dt, isOutput=True)

    relu = mybir.ActivationFunctionType.Relu
    alu_add = mybir.AluOpType.add
    alu_max = mybir.AluOpType.max

    # stage-1/2 m-halves whose drain runs on ACT (rest on DVE)
    s12_act = {
        int(x)
        for x in os.environ.get("BOT_S12_ACT", "0").split(",")
        if x != ""
    }
    # stage-3 m-groups whose drain runs on ACT (rest on DVE)
    s3_act = {
        int(x)
        for x in os.environ.get("BOT_S3_ACT", "1,5").split(",")
        if x != ""
    }
    # stage-3 m-groups whose output DMA rides the ACT HWDGE queue
    out_act = {
        int(x)
        for x in os.environ.get("BOT_OUT_ACT", "6,7").split(",")
        if x != ""
    }

    def post(on_act, dst, src, bias_ap):
        # relu(src + bias) -> dst on the chosen engine
        if on_act:
            nc.scalar.activation(dst, src, relu, bias=bias_ap)
        else:
            nc.vector.tensor_scalar(dst, src, bias_ap, 0.0, alu_add, alu_max)

    import contextlib

    with tile.TileContext(nc) as tc:
        with (
            tc.tile_pool(name="consts", bufs=2) as consts,
            tc.tile_pool(name="psum", bufs=8, space="PSUM") as psum,
            tc.tile_pool(name="outp", bufs=3) as outp,
            (
                tc.For_i(0, loop_n, 1, hint_engines=(mybir.EngineType.PE,),
                         staggered_reset=True)
                if loop_n
                else contextlib.nullcontext()
            ),
        ):
            for _rep in range(reps):
                # --- SP queue: w1 first (unblocks the PE), then the xs
                # chunks in consumption order, then the small biases.
                # A pure-dma_start stream: next iteration's loads issue
                # as soon as pool buffers free.
                w1_t = consts.tile([128, 4 * 256], act_dt, tag="w1")
                nc.sync.dma_start(
                    out=w1_t.rearrange("p (k c) -> p k c", k=4),
                    in_=w1_d.rearrange("(k p) c -> p k c", p=128),
                )
                w1_sb = [w1_t[:, k * 256:(k + 1) * 256] for k in range(4)]
                b1_sb = consts.tile([128, 2], F32, tag="b1")
                nc.sync.dma_start(out=b1_sb, in_=b1_d[:, :])
                xs_sb = [
                    consts.tile([128, PB], act_dt, tag=f"xs_{k}", name=f"xs_{k}")
                    for k in range(4)
                ]
                for k in range(4):
                    nc.sync.dma_start(
                        out=xs_sb[k], in_=xs_d[k * 128:(k + 1) * 128, :]
                    )
                b2_sb = consts.tile([128, 2], F32, tag="b2")
                nc.sync.dma_start(out=b2_sb, in_=b2_d[:, :])
                b34_sb = consts.tile([128, 8], F32, tag="b34")
                nc.sync.dma_start(out=b34_sb, in_=b34_d[:, :])

                # --- ACT queue: w2/w3/w4, needed from ~10us in ---
                w2_t = consts.tile([128, 18 * 256], act_dt, tag="w2")
                nc.scalar.dma_start(
                    out=w2_t.rearrange("p (c n) -> p c n", c=18),
                    in_=w2_d.rearrange("(c p) n -> p c n", p=128),
                )
                w2_sb = [
                    [
                        w2_t[:, (tap * 2 + k) * 256:(tap * 2 + k + 1) * 256]
                        for k in range(2)
                    ]
                    for tap in range(9)
                ]
                w3_t = consts.tile([128, 2 * 1024], act_dt, tag="w3")
                nc.scalar.dma_start(
                    out=w3_t.rearrange("p (k c) -> p k c", k=2),
                    in_=w3_d.rearrange("(k p) c -> p k c", p=128),
                )
                w3_sb = [w3_t[:, k * 1024:(k + 1) * 1024] for k in range(2)]
                w4_t = consts.tile([128, 4 * 1024], act_dt, tag="w4")
                nc.scalar.dma_start(
                    out=w4_t.rearrange("p (k c) -> p k c", k=4),
                    in_=w4_d.rearrange("(k p) c -> p k c", p=128),
                )
                w4_sb = [w4_t[:, k * 1024:(k + 1) * 1024] for k in range(4)]

                # --- t1 padded planes: zero only the halo border ---
                t1pad = []
                for k in range(2):
                    t = consts.tile([128, B * PADQ], act_dt, tag=f"t1p_{k}")
                    if os.environ.get("BOT_MEMSET", "border") == "full":
                        nc.vector.memset(t, 0.0)
                    else:
                        pl = t.rearrange("p (i q) -> p i q", i=B)
                        # rows 15..16 of every image
                        nc.vector.memset(pl[:, :, 15 * WP:], 0.0)
                        # row 0 of every image
                        nc.vector.memset(pl[:, :, 0:WP], 0.0)
                        # cols 15,0-of-next-row for rows 1..14
                        nc.vector.memset(
                            pl.rearrange("p i (h w) -> p i h w", w=WP)[
                                :, :, 1:15, 15:16
                            ],
                            0.0,
                        )
                        nc.vector.memset(
                            pl.rearrange("p i (h w) -> p i h w", w=WP)[
                                :, :, 1:15, 0:1
                            ],
                            0.0,
                        )
                    t1pad.append(t)
                t2_sb = []
                for k in range(2):
                    t = consts.tile([128, PB], act_dt, tag=f"t2_{k}")
                    t2_sb.append(t)

                # --- stage 1: conv1 + relu, scattered into padded planes.
                # k-outer: each arriving xs chunk feeds 8 matmuls (both
                # output halves), PSUM accumulates across chunks.
                for m in range(2 if STAGES >= 1 else 0):
                    pss = [psum.tile([128, NF], F32, tag="ps", name=f"ps{_g}")
                           for _g in range(G)]
                    for k in range(4):
                        for g in range(G):
                            nc.tensor.matmul(
                                pss[g][:, :],
                                w1_sb[k][:, m * 128:(m + 1) * 128],
                                xs_sb[k][:, g * NF:(g + 1) * NF],
                                start=(k == 0),
                                stop=(k == 3),
                            )
                    for g in range(G):
                        dst = t1pad[m][
                            :, g * NG * PADQ:(g * NG + NG) * PADQ
                        ].rearrange("p (n h w) -> p n h w", h=17, w=WP)[
                            :, :, 1:15, 1:15
                        ]
                        post((m * 4 + g) % 2 == 0, dst, pss[g][:, :],
                             b1_sb[:, m:m + 1])

                # --- stage 2: conv2 (3x3 as 9 shifted matmuls) + relu ---
                # weight-stationary: each tap/k weight feeds all 4 image
                # groups; moving operand is a 4-level shifted view of the
                # padded planes so pad columns are never streamed
                for m in range(2 if STAGES >= 2 else 0):
                    pss = [psum.tile([128, NF], F32, tag="ps", name=f"ps{_g}")
                           for _g in range(G)]
                    i = 0
                    for tap in range(9):
                        dy, dx = divmod(tap, 3)
                        for k in range(2):
                            for g in range(G):
                                seg = t1pad[k][
                                    :, g * NG * PADQ:(g * NG + NG) * PADQ
                                ].rearrange(
                                    "p (n h w) -> p n h w", h=17, w=WP
                                )[:, :, dy:dy + HW, dx:dx + HW]
                                nc.tensor.matmul(
                                    pss[g][:, :],
                                    w2_sb[tap][k][:, m * 128:(m + 1) * 128],
                                    seg,
                                    start=(i == 0),
                                    stop=(i == 17),
                                )
                            i += 1
                    for g in range(G):
                        post((m * 4 + g) % 2 == 0,
                             t2_sb[m][:, g * NF:(g + 1) * NF],
                             pss[g][:, :], b2_sb[:, m:m + 1])

                # --- stage 3: conv3 + residual conv4 in one PSUM, relu ---
                for m in range(8 if STAGES >= 3 else 0):
                    pss = [psum.tile([128, NF], F32, tag="ps", name=f"ps{_g}")
                           for _g in range(G)]
                    # conv4 first: it depends only on xs, so its matmuls can
                    # fill the PE while stage 2 still drains
                    for k in range(4):
                        for g in range(G):
                            nc.tensor.matmul(
                                pss[g][:, :],
                                w4_sb[k][:, m * 128:(m + 1) * 128],
                                xs_sb[k][:, g * NF:(g + 1) * NF],
                                start=(k == 0),
                                stop=False,
                            )
                    for k in range(2):
                        for g in range(G):
                            nc.tensor.matmul(
                                pss[g][:, :],
                                w3_sb[k][:, m * 128:(m + 1) * 128],
                                t2_sb[k][:, g * NF:(g + 1) * NF],
                                start=False,
                                stop=(k == 1),
                            )
                    ot = outp.tile([128, PB], out_dt, tag="ot")
                    for g in range(G):
                        post(g == 3, ot[:, g * NF:(g + 1) * NF],
                             pss[g][:, :], b34_sb[:, m:m + 1])
                    if m in out_act:
                        nc.scalar.dma_start(
                            out=out_d[m * 128:(m + 1) * 128, :], in_=ot,
                        )
                    else:
                        nc.gpsimd.dma_start(
                            out=out_d[m * 128:(m + 1) * 128, :], in_=ot,
                        )
                if STAGES < 3:
                    ot = outp.tile([128, PB], out_dt, tag="ot")
                    src_t = (t2_sb[0] if STAGES >= 2 else
                             (t1pad[0][:, 0:PB] if STAGES >= 1
                              else xs_sb[0][:, 0:PB]))
                    nc.vector.tensor_copy(ot, src_t)
                    nc.gpsimd.dma_start(out=out_d[0:128, :], in_=ot)
    nc.finalize()
    if os.environ.get("BOT_LDW_DEDUP", "1") == "1":
        n = _dedupe_ldweights(nc)
        if os.environ.get("BOT_VERBOSE"):
            print(f"[kernel] deduped {n} InstLdweights")
    if os.environ.get("BOT_SEM_THIN", "1") == "1":
        n = _thin_pe_sem_updates(nc)
        if os.environ.get("BOT_VERBOSE"):
            print(f"[kernel] thinned {n} matmul sem updates")
    if os.environ.get("BOT_WAIT_STRIP", "1") == "1":
        n = _strip_redundant_waits(nc)
        if os.environ.get("BOT_VERBOSE"):
            print(f"[kernel] stripped {n} redundant waits")
    return nc


def _prep(x, w1, w2, w3, w4, s1, b1, s2, b2, s3, b3, s4, b4):
    """Host-side input prep: shard, fold BN, transpose. All numpy."""
    if COMPUTE_DT == "bf16":
        import ml_dtypes

        cdt = np.dtype(ml_dtypes.bfloat16)
    else:
        cdt = np.dtype(np.float32)

    # x -> even positions, (core, c, n, h*14+w) channel-major partition lines
    xs = x[:, :, ::2, ::2].reshape(N_CORES, B, 512, P).transpose(0, 2, 1, 3)
    xs = np.ascontiguousarray(xs).reshape(N_CORES, 512, PB).astype(cdt)

    w1f = (w1[:, :, 0, 0] * s1[:, None]).T                    # (512, 256)
    w2f = w2 * s2[:, None, None, None]                        # (256,256,3,3)
    w2t = np.stack(
        [w2f[:, :, dy, dx].T for dy in range(3) for dx in range(3)]
    ).reshape(9 * 256, 256)                                   # (2304, 256)
    w3f = (w3[:, :, 0, 0] * s3[:, None]).T                    # (256, 1024)
    w4f = (w4[:, :, 0, 0] * s4[:, None]).T                    # (512, 1024)

    com = {
        "w1t": np.ascontiguousarray(w1f).astype(cdt),
        "w2t": np.ascontiguousarray(w2t).astype(cdt),
        "w3t": np.ascontiguousarray(w3f).astype(cdt),
        "w4t": np.ascontiguousarray(w4f).astype(cdt),
        "b1p": np.ascontiguousarray(b1.reshape(2, 128).T).astype(np.float32),
        "b2p": np.ascontiguousarray(b2.reshape(2, 128).T).astype(np.float32),
        "b34p": np.ascontiguousarray(
            (b3 + b4).reshape(8, 128).T
        ).astype(np.float32),
    }
    return [{"xs": xs[c], **com} for c in range(N_CORES)]


def _gather(results):
    out = np.empty((64, 1024, HW, HW), np.float32)
    for c, r in enumerate(results):
        o = r["out"].astype(np.float32).reshape(1024, B, HW, HW)
        out[c * B:(c + 1) * B] = o.transpose(1, 0, 2, 3)
    return out


def _get_nc(reps=1, loop_n=0):
    key = ("nc", reps, loop_n)
    if key not in _CACHE:
        _CACHE[key] = _build_nc(reps, loop_n)
    return _CACHE[key]


def _run(in_maps, **kwargs):
    return run_bass_kernel_spmd(
        _get_nc(), in_maps, list(range(N_CORES)), **kwargs
    )


def kernel(**inputs):
    in_maps = _prep(**inputs)
    res = _run(in_maps)
    return _gather(res.results)


def _pjrt_runner(nc, in_maps):
    """Compile nc once; return (run_once, run_batch, results).

    run_once(): one blocking execution. run_batch(n): n pipelined
    executions, blocking at the end; returns elapsed seconds. results:
    first run's outputs as a list of per-core dicts.
    """
    import time

    import jax
    import numpy as np_
    from jax.sharding import Mesh, NamedSharding, PartitionSpec
    from jax.experimental.shard_map import shard_map

    from concourse import bass2jax, mybir as mb

    bass2jax.install_neuronx_cc_hook()
    part_name = nc.partition_id_tensor.name if nc.partition_id_tensor else None
    in_names, out_names, out_avals = [], [], []
    for alloc in nc.m.functions[0].allocations:
        if not isinstance(alloc, mb.MemoryLocationSet):
            continue
        name = alloc.memorylocations[0].name
        if alloc.kind == "ExternalInput":
            if name != part_name:
                in_names.append(name)
        elif alloc.kind == "ExternalOutput":
            out_names.append(name)
            out_avals.append(
                jax.core.ShapedArray(
                    tuple(alloc.tensor_shape), mb.dt.np(alloc.dtype)
                )
            )
    all_names = in_names + out_names + ([part_name] if part_name else [])

    def _body(*args):
        operands = list(args)
        if part_name is not None:
            operands.append(bass2jax.partition_id_tensor())
        outs = bass2jax._bass_exec_p.bind(
            *operands,
            out_avals=tuple(out_avals),
            in_names=tuple(all_names),
            out_names=tuple(out_names),
            lowering_input_output_aliases=(),
            sim_require_finite=False,
            sim_require_nnan=False,
            nc=nc,
        )
        return tuple(outs)

    devices = jax.devices()[:N_CORES]
    mesh = Mesh(np_.asarray(devices), ("core",))
    nspec = len(in_names) + len(out_names)
    sharded = jax.jit(
        shard_map(
            _body,
            mesh=mesh,
            in_specs=(PartitionSpec("core"),) * nspec,
            out_specs=(PartitionSpec("core"),) * len(out_names),
            check_rep=False,
        ),
        keep_unused=True,
    )

    sh = NamedSharding(mesh, PartitionSpec("core"))
    dev_args = [
        jax.device_put(
            np_.concatenate([in_maps[c][n] for c in range(N_CORES)], axis=0), sh
        )
        for n in in_names
    ] + [
        jax.device_put(
            np_.zeros((N_CORES * a.shape[0], *a.shape[1:]), a.dtype), sh
        )
        for a in out_avals
    ]

    outs = jax.block_until_ready(sharded(*dev_args))  # compile + warm

    results = [
        {
            n: np_.asarray(outs[i]).reshape(N_CORES, *out_avals[i].shape)[c]
            for i, n in enumerate(out_names)
        }
        for c in range(N_CORES)
    ]

    def run_once():
        jax.block_until_ready(sharded(*dev_args))

    def run_batch(n):
        t0 = time.monotonic()
        r = None
        for _ in range(n):
            r = sharded(*dev_args)
        jax.block_until_ready(r)
        return time.monotonic() - t0

    return run_once, run_batch, results


def kernel_timed(**inputs):
    """Run + estimate steady-state per-execution device time (ns).

    NTFF profiling is unavailable under this axon client and per-call
    wall time is ~80ms of tunnel overhead, so device time is measured
    with a hardware For_i loop: two NEFFs run the 2-rep body loop_n1 /
    loop_n2 times; the wall-time slope over (loop_n2-loop_n1)*2 reps
    cancels the per-call overhead.  Returns (out, exec_time_ns).
    """
    import time

    import numpy as np_

    n1 = int(os.environ.get("BOT_LOOP_N1", "16"))
    n2 = int(os.environ.get("BOT_LOOP_N2", "112"))
    reps = int(os.environ.get("BOT_LOOP_REPS", "4"))
    trials = int(os.environ.get("BOT_LOOP_TRIALS", "40"))
    in_maps = _prep(**inputs)

    res = _run(in_maps)
    out = _gather(res.results)

    once1, _, _ = _pjrt_runner(_get_nc(reps, loop_n=n1), in_maps)
    once2, _, _ = _pjrt_runner(_get_nc(reps, loop_n=n2), in_maps)

    def s(f):
        t0 = time.monotonic()
        f()
        return time.monotonic() - t0

    for _ in range(2):
        s(once1), s(once2)
    t1s, t2s = [], []
    for _ in range(trials):
        t1s.append(s(once1))
        t2s.append(s(once2))
    t1s, t2s = np_.array(t1s), np_.array(t2s)
    per = (t2s - t1s) / (n2 - n1) / reps * 1e9
    per_rep = int(np_.median(per))
    print(f"[bench] For_i loop slope over {trials} trials: {per_rep} ns "
          f"(iqr {np_.percentile(per, 25):.0f}-{np_.percentile(per, 75):.0f})")
    return out, per_rep


# revision 7
# speedup vs baseline: 1.1541x; 1.0292x over previous
"""ResNet bottleneck block (dense_cnn) on 8 Trainium2 NeuronCores.

Reference computation (NCHW, fp32):
    t1  = relu(s1 * conv1x1(x, w1, stride=2) + b1)     # 512 -> 256, 28x28 -> 14x14
    t2  = relu(s2 * conv3x3(t1, w2, pad=1)   + b2)     # 256 -> 256
    t3  =      s3 * conv1x1(t2, w3)          + b3      # 256 -> 1024
    idn =      s4 * conv1x1(x, w4, stride=2) + b4      # 512 -> 1024
    out = relu(t3 + idn)                               # (64, 1024, 14, 14)

Strategy:
  - Data-parallel over batch: 64 images -> 8 cores x 8 images.
  - Host-side prep (numpy, cheap): subsample x to its even (h, w)
    positions, fold BN scales into conv weights, transpose weights to
    [ci, co], cast to bf16 (rel err ~5e-3 << the 2e-2 gate at ~2x the
    PE/DMA throughput of fp32 paths).
  - On-chip: every conv is a matmul with channels on partitions and
    (image, h, w) on the free dim (392 columns = 2 images).  The 3x3
    conv is 9 shifted matmuls accumulating in PSUM over zero-padded
    16-wide planes of t1.  Weight-stationary ordering amortizes
    LDWEIGHTS (hidden by the PE reorder window).
  - PSUM: each half-stage owns a [128, 2048]-f32 tile (4 banks); the 4
    image groups live at 512-col bank-aligned offsets so a whole
    m-group drains with ONE wide ACT/DVE instruction (bias+relu fused)
    instead of 4-16 narrow ones; engine fixed overhead (~0.2us/instr)
    amortizes 4-8x.
  - Stage 1 is k-outer (both conv1 output halves accumulate in PSUM
    across arriving xs chunks) so each 0.4MB xs chunk unlocks 8
    matmuls and the input stream stays ahead of the PE.
  - Stage 3 issues the conv4 (residual, xs-only) matmuls before conv3
    so the PE fills while stage 2 still drains; conv3 accumulates into
    the same PSUM so the add + final relu are one pass.
  - DMA queues: SP-HWDGE carries w1+xs(+biases) -- a pure-dma_start
    instruction stream, so the next For_i iteration's loads prefetch as
    soon as buffers free instead of queueing behind drain work.
    ACT-HWDGE carries w2/w3/w4 (not needed until ~10us in) and the last
    two output chunks; the Pool-engine SWDGE streams the rest of the
    output.  t1pad zeroing only touches the halo border (~900 cols vs
    4352 for full planes).
  - Post-finalize IR passes: drop InstLdweights duplicated by
    legalization for repeated stationary operands, thin matmul
    semaphore updates to the waited counts, and strip semaphore waits
    already implied by earlier waits on the same engine.

Measured per-execution device time via a hardware For_i loop slope:
~66us on the axon-tunneled trn2 (baseline of this session: ~80us).
Pure-PE floor for 368 matmuls of 392 cols at the measured back-to-back
rate (24ns + 0.52ns/col under full-power P0 clocks) is ~64us.
"""

import os

import numpy as np

import concourse.mybir as mybir
import concourse.tile as tile
from concourse import bacc
from concourse.bass_utils import run_bass_kernel_spmd

F32 = mybir.dt.float32
BF16 = mybir.dt.bfloat16
F32R = mybir.dt.float32r
I32 = mybir.dt.int32

N_CORES = 8
B = 8              # images per core
HW = 14            # output spatial
P = HW * HW        # 196 per image plane (compact)
PB = B * P         # 1568
WP = 16            # padded row width for the 3x3 conv input
Q = HW * WP        # 224 (padded-plane columns per image in conv2 psum)
PADQ = 17 * WP     # 272 per-image padded plane (1 extra slack row)
NG = 2             # images per matmul group
G = B // NG        # 4 groups
NF = NG * P        # 392: compact moving-operand free size
GP = 512           # psum columns per image group (bank-aligned)

# Compute dtype for matmuls: "f32r" (fp32 storage, TF32-like multiply,
# full PE rate), "f32" (exact, 1/4 rate), "bf16".
COMPUTE_DT = os.environ.get("BOT_DT", "bf16")
# Debug: build only the first N stages (1..3) for per-stage HW timing.
STAGES = int(os.environ.get("BOT_STAGES", "3"))

_CACHE = {}


def _dedupe_ldweights(nc):
    """Drop InstLdweights identical to the previous one in the PE stream.

    Legalization emits one weight load per matmul; when consecutive
    matmuls share the stationary operand the repeated ~107ns loads are
    pure PE overhead.  Only waits/updates-free duplicates are dropped,
    so semaphore counts are unchanged.
    """

    def ap_sig(ap):
        try:
            ml = ap.memorylocation
            name = ml.name if ml is not None else None
        except Exception:
            name = None
        off = getattr(ap, "offset", None)
        try:
            dims = tuple((d.num_elem, d.step) for d in ap.aps)
        except Exception:
            dims = str(ap)
        return (name, off, dims)

    removed = 0
    for fn in nc.m.functions:
        for blk in fn.blocks:
            insts = blk.instructions
            last_sig = None
            keep = []
            for inst in insts:
                if isinstance(inst, mybir.InstLdweights):
                    sig = (
                        ap_sig(inst.ins[0]),
                        getattr(inst, "perf_mode", None),
                        getattr(inst, "is_transpose", None),
                        getattr(inst, "tile_position", None),
                    )
                    si = inst.sync_info
                    clean = si is None or (
                        len(si.on_wait) == 0 and len(si.on_update) == 0
                    )
                    if clean and sig == last_sig:
                        removed += 1
                        continue
                    last_sig = sig
                elif not isinstance(inst, mybir.InstMatmult):
                    if isinstance(
                        inst,
                        (mybir.InstUnconditionalBranch, mybir.InstCall),
                    ):
                        last_sig = None
                keep.append(inst)
            if len(keep) != len(insts):
                del insts[:]
                insts.extend(keep)
    return removed


def _strip_redundant_waits(nc):
    """Remove semaphore waits already implied by earlier waits.

    Engines execute their instruction stream in order, so once an
    instruction on engine E has waited for sem >= v, every later wait on
    E for sem >= v' with v' <= v is a no-op.  Each retired wait still
    costs the sequencer dispatch time, so stripping them shortens the
    per-instruction issue path.  Tracking is per block and resets at
    event-semaphore (barrier/reset) instructions, which is conservative
    for loop back-edges.
    """
    removed = 0
    for fn in nc.m.functions:
        for blk in fn.blocks:
            seen = {}  # (engine, sem_id) -> max value waited
            for inst in blk.instructions:
                if isinstance(inst, mybir.InstEventSemaphore):
                    seen = {k: v for k, v in seen.items() if k[0] != inst.engine}
                    # barriers also imply cross-engine sync; be safe:
                    seen = {}
                    continue
                si = inst.sync_info
                if si is None or not si.on_wait:
                    continue
                kept = []
                for w in si.on_wait:
                    if (
                        getattr(w, "sync_type", None) == "semaphore"
                        and getattr(w, "wait_mode", None) == "sem-ge-imm"
                        and getattr(w, "wait_value", None) is not None
                    ):
                        key = (inst.engine, w.id)
                        if seen.get(key, -1) >= w.wait_value:
                            removed += 1
                            continue
                        seen[key] = w.wait_value
                    kept.append(w)
                if len(kept) != len(si.on_wait):
                    si.on_wait = kept
    return removed


def _thin_pe_sem_updates(nc):
    """Drop matmul semaphore increments no consumer distinguishes.

    Every matmul increments the PE progress semaphore (~26ns serialized
    EVT_SEM write each), but consumers wait on only a few distinct
    counts.  Keep exactly the increments at waited cumulative counts and
    renumber every wait to its rank among kept values.  Applied only to
    semaphores whose updates are exclusively matmul sem-inc(+1) and
    whose waits are all sem-ge-imm, so semantics are preserved.
    """
    removed = 0
    for fn in nc.m.functions:
        upd, bad, waits = {}, set(), {}
        for blk in fn.blocks:
            for inst in blk.instructions:
                si = inst.sync_info
                if not si:
                    continue
                for u in si.on_update:
                    if getattr(u, "sync_type", None) != "semaphore":
                        continue
                    if (
                        getattr(u, "update_mode", None) != "sem-inc"
                        or getattr(u, "update_value", None) != 1
                        or not isinstance(inst, mybir.InstMatmult)
                    ):
                        bad.add(u.id)
                    upd.setdefault(u.id, []).append((inst, u))
                for w in si.on_wait:
                    if getattr(w, "sync_type", None) != "semaphore":
                        continue
                    if (
                        getattr(w, "wait_mode", None) != "sem-ge-imm"
                        or getattr(w, "wait_value", None) is None
                    ):
                        bad.add(getattr(w, "id", None))
                        continue
                    waits.setdefault(w.id, []).append(w)
        for sid, entries in upd.items():
            if sid in bad or len(entries) < 8:
                continue
            vals = sorted({w.wait_value for w in waits.get(sid, [])})
            pos = [v for v in vals if v >= 1]
            if not pos or pos[-1] > len(entries):
                continue
            keep = set(pos)
            rank = {v: i + 1 for i, v in enumerate(pos)}
            for idx, (inst, u) in enumerate(entries):
                if idx + 1 not in keep:
                    inst.sync_info.on_update = [
                        x for x in inst.sync_info.on_update if x is not u
                    ]
                    removed += 1
            for w in waits.get(sid, []):
                if w.wait_value >= 1:
                    w.wait_value = rank[w.wait_value]
    return removed


def _build_nc(reps=1, loop_n=0):
    """loop_n > 0 wraps the body in a hardware For_i loop (timing only)."""
    act_dt = {"bf16": BF16, "f32": F32, "f32r": F32R}[COMPUTE_DT]
    # bf16 output halves the out DMA and the drain write traffic; host
    # upcasts after gather.  Error stays ~1e-3 << the 2e-2 gate.
    out_dt = BF16 if (act_dt == BF16
                      and os.environ.get("BOT_OUT_BF16", "1") == "1") else F32

    nc = bacc.Bacc()
    xs_d = nc.declare_dram_parameter("xs", [512, PB], act_dt, isOutput=False)
    w1_d = nc.declare_dram_parameter("w1t", [512, 256], act_dt, isOutput=False)
    w2_d = nc.declare_dram_parameter("w2t", [9 * 256, 256], act_dt, isOutput=False)
    w3_d = nc.declare_dram_parameter("w3t", [256, 1024], act_dt, isOutput=False)
    w4_d = nc.declare_dram_parameter("w4t", [512, 1024], act_dt, isOutput=False)
    b1_d = nc.declare_dram_parameter("b1p", [128, 2], F32, isOutput=False)
    b2_d = nc.declare_dram_parameter("b2p", [128, 2], F32, isOutput=False)
    b34_d = nc.declare_dram_parameter("b34p", [128, 8], F32, isOutput=False)
    out_d = nc.declare_dram_parameter("out", [1024, PB], out_dt, isOutput=True)

    relu = mybir.ActivationFunctionType.Relu
    alu_add = mybir.AluOpType.add
    alu_max = mybir.AluOpType.max

    # stage-1/2 m-halves whose drain runs on ACT (rest on DVE)
    s12_act = {
        int(x)
        for x in os.environ.get("BOT_S12_ACT", "0").split(",")
        if x != ""
    }
    # stage-3 m-groups whose drain runs on ACT (rest on DVE)
    s3_act = {
        int(x)
        for x in os.environ.get("BOT_S3_ACT", "1,5").split(",")
        if x != ""
    }
    # stage-3 m-groups whose output DMA rides the ACT HWDGE queue
    out_act = {
        int(x)
        for x in os.environ.get("BOT_OUT_ACT", "6,7").split(",")
        if x != ""
    }

    def post(on_act, dst, src, bias_ap):
        # relu(src + bias) -> dst on the chosen engine
        if on_act:
            nc.scalar.activation(dst, src, relu, bias=bias_ap)
        else:
            nc.vector.tensor_scalar(dst, src, bias_ap, 0.0, alu_add, alu_max)

    import contextlib

    with tile.TileContext(nc) as tc:
        with (
            tc.tile_pool(name="consts", bufs=2) as consts,
            tc.tile_pool(name="psum", bufs=8, space="PSUM") as psum,
            tc.tile_pool(name="outp", bufs=3) as outp,
            (
                tc.For_i(0, loop_n, 1, hint_engines=(mybir.EngineType.PE,),
                         staggered_reset=True)
                if loop_n
                else contextlib.nullcontext()
            ),
        ):
            for _rep in range(reps):
                # --- SP queue: w1 first (unblocks the PE), then the xs
                # chunks in consumption order, then the small biases.
                # A pure-dma_start stream: next iteration's loads issue
                # as soon as pool buffers free.
                w1_t = consts.tile([128, 4 * 256], act_dt, tag="w1")
                nc.sync.dma_start(
                    out=w1_t.rearrange("p (k c) -> p k c", k=4),
                    in_=w1_d.rearrange("(k p) c -> p k c", p=128),
                )
                w1_sb = [w1_t[:, k * 256:(k + 1) * 256] for k in range(4)]
                b1_sb = consts.tile([128, 2], F32, tag="b1")
                nc.sync.dma_start(out=b1_sb, in_=b1_d[:, :])
                xs_sb = [
                    consts.tile([128, PB], act_dt, tag=f"xs_{k}", name=f"xs_{k}")
                    for k in range(4)
                ]
                for k in range(4):
                    nc.sync.dma_start(
                        out=xs_sb[k], in_=xs_d[k * 128:(k + 1) * 128, :]
                    )
                b2_sb = consts.tile([128, 2], F32, tag="b2")
                nc.sync.dma_start(out=b2_sb, in_=b2_d[:, :])
                b34_sb = consts.tile([128, 8], F32, tag="b34")
                nc.sync.dma_start(out=b34_sb, in_=b34_d[:, :])

                # --- ACT queue: w2/w3/w4, needed from ~10us in ---
                w2_t = consts.tile([128, 18 * 256], act_dt, tag="w2")
                nc.scalar.dma_start(
                    out=w2_t.rearrange("p (c n) -> p c n", c=18),
                    in_=w2_d.rearrange("(c p) n -> p c n", p=128),
                )
                w2_sb = [
                    [
                        w2_t[:, (tap * 2 + k) * 256:(tap * 2 + k + 1) * 256]
                        for k in range(2)
                    ]
                    for tap in range(9)
                ]
                w3_t = consts.tile([128, 2 * 1024], act_dt, tag="w3")
                nc.scalar.dma_start(
                    out=w3_t.rearrange("p (k c) -> p k c", k=2),
                    in_=w3_d.rearrange("(k p) c -> p k c", p=128),
                )
                w3_sb = [w3_t[:, k * 1024:(k + 1) * 1024] for k in range(2)]
                w4_t = consts.tile([128, 4 * 1024], act_dt, tag="w4")
                nc.scalar.dma_start(
                    out=w4_t.rearrange("p (k c) -> p k c", k=4),
                    in_=w4_d.rearrange("(k p) c -> p k c", p=128),
                )
                w4_sb = [w4_t[:, k * 1024:(k + 1) * 1024] for k in range(4)]

                # --- t1 padded planes: zero only the halo border ---
                t1pad = []
                for k in range(2):
                    t = consts.tile([128, B * PADQ], act_dt, tag=f"t1p_{k}")
                    if os.environ.get("BOT_MEMSET", "border") == "full":
                        nc.vector.memset(t, 0.0)
                    else:
                        pl = t.rearrange("p (i q) -> p i q", i=B)
                        # rows 15..16 of every image
                        nc.vector.memset(pl[:, :, 15 * WP:], 0.0)
                        # row 0 of every image
                        nc.vector.memset(pl[:, :, 0:WP], 0.0)
                        # cols 15,0-of-next-row for rows 1..14
                        nc.vector.memset(
                            pl.rearrange("p i (h w) -> p i h w", w=WP)[
                                :, :, 1:15, 15:16
                            ],
                            0.0,
                        )
                        nc.vector.memset(
                            pl.rearrange("p i (h w) -> p i h w", w=WP)[
                                :, :, 1:15, 0:1
                            ],
                            0.0,
                        )
                    t1pad.append(t)
                t2_sb = []
                for k in range(2):
                    t = consts.tile([128, PB], act_dt, tag=f"t2_{k}")
                    t2_sb.append(t)

                # --- stage 1: conv1 + relu, scattered into padded planes.
                # k-outer: each arriving xs chunk feeds 8 matmuls (both
                # output halves), PSUM accumulates across chunks.
                for m in range(2 if STAGES >= 1 else 0):
                    pss = [psum.tile([128, NF], F32, tag="ps", name=f"ps{_g}")
                           for _g in range(G)]
                    for k in range(4):
                        for g in range(G):
                            nc.tensor.matmul(
                                pss[g][:, :],
                                w1_sb[k][:, m * 128:(m + 1) * 128],
                                xs_sb[k][:, g * NF:(g + 1) * NF],
                                start=(k == 0),
                                stop=(k == 3),
                            )
                    for g in range(G):
                        dst = t1pad[m][
                            :, g * NG * PADQ:(g * NG + NG) * PADQ
                        ].rearrange("p (n h w) -> p n h w", h=17, w=WP)[
                            :, :, 1:15, 1:15
                        ]
                        post((m * 4 + g) % 2 == 0, dst, pss[g][:, :],
                             b1_sb[:, m:m + 1])

                # --- stage 2: conv2 (3x3 as 9 shifted matmuls) + relu ---
                # weight-stationary: each tap/k weight feeds all 4 image
                # groups; moving operand is a 4-level shifted view of the
                # padded planes so pad columns are never streamed
                for m in range(2 if STAGES >= 2 else 0):
                    pss = [psum.tile([128, NF], F32, tag="ps", name=f"ps{_g}")
                           for _g in range(G)]
                    i = 0
                    for tap in range(9):
                        dy, dx = divmod(tap, 3)
                        for k in range(2):
                            for g in range(G):
                                seg = t1pad[k][
                                    :, g * NG * PADQ:(g * NG + NG) * PADQ
                                ].rearrange(
                                    "p (n h w) -> p n h w", h=17, w=WP
                                )[:, :, dy:dy + HW, dx:dx + HW]
                                nc.tensor.matmul(
                                    pss[g][:, :],
                                    w2_sb[tap][k][:, m * 128:(m + 1) * 128],
                                    seg,
                                    start=(i == 0),
                                    stop=(i == 17),
                                )
                            i += 1
                    for g in range(G):
                        post((m * 4 + g) % 2 == 0,
                             t2_sb[m][:, g * NF:(g + 1) * NF],
                             pss[g][:, :], b2_sb[:, m:m + 1])

                # --- stage 3: conv3 + residual conv4 in one PSUM, relu ---
                for m in range(8 if STAGES >= 3 else 0):
                    pss = [psum.tile([128, NF], F32, tag="ps", name=f"ps{_g}")
                           for _g in range(G)]
                    # conv4 first: it depends only on xs, so its matmuls can
                    # fill the PE while stage 2 still drains
                    for k in range(4):
                        for g in range(G):
                            nc.tensor.matmul(
                                pss[g][:, :],
                                w4_sb[k][:, m * 128:(m + 1) * 128],
                                xs_sb[k][:, g * NF:(g + 1) * NF],
                                start=(k == 0),
                                stop=False,
                            )
                    for k in range(2):
                        for g in range(G):
                            nc.tensor.matmul(
                                pss[g][:, :],
                                w3_sb[k][:, m * 128:(m + 1) * 128],
                                t2_sb[k][:, g * NF:(g + 1) * NF],
                                start=False,
                                stop=(k == 1),
                            )
                    ot = outp.tile([128, PB], out_dt, tag="ot")
                    for g in range(G):
                        post(g == 3, ot[:, g * NF:(g + 1) * NF],
                             pss[g][:, :], b34_sb[:, m:m + 1])
                    if m in out_act:
                        nc.scalar.dma_start(
                            out=out_d[m * 128:(m + 1) * 128, :], in_=ot,
                        )
                    else:
                        nc.gpsimd.dma_start(
                            out=out_d[m * 128:(m + 1) * 128, :], in_=ot,
                        )
                if STAGES < 3:
                    ot = outp.tile([128, PB], out_dt, tag="ot")
                    src_t = (t2_sb[0] if STAGES >= 2 else
                             (t1pad[0][:, 0:PB] if STAGES >= 1
                              else xs_sb[0][:, 0:PB]))
                    nc.vector.tensor_copy(ot, src_t)
                    nc.gpsimd.dma_start(out=out_d[0:128, :], in_=ot)
    nc.finalize()
    if os.environ.get("BOT_LDW_DEDUP", "1") == "1":
        n = _dedupe_ldweights(nc)
        if os.environ.get("BOT_VERBOSE"):
            print(f"[kernel] deduped {n} InstLdweights")
    if os.environ.get("BOT_SEM_THIN", "1") == "1":
        n = _thin_pe_sem_updates(nc)
        if os.environ.get("BOT_VERBOSE"):
            print(f"[kernel] thinned {n} matmul sem updates")
    if os.environ.get("BOT_WAIT_STRIP", "1") == "1":
        n = _strip_redundant_waits(nc)
        if os.environ.get("BOT_VERBOSE"):
            print(f"[kernel] stripped {n} redundant waits")
    return nc


def _prep(x, w1, w2, w3, w4, s1, b1, s2, b2, s3, b3, s4, b4):
    """Host-side input prep: shard, fold BN, transpose. All numpy."""
    if COMPUTE_DT == "bf16":
        import ml_dtypes

        cdt = np.dtype(ml_dtypes.bfloat16)
    else:
        cdt = np.dtype(np.float32)

    # x -> even positions, (core, c, n, h*14+w) channel-major partition lines
    xs = x[:, :, ::2, ::2].reshape(N_CORES, B, 512, P).transpose(0, 2, 1, 3)
    xs = np.ascontiguousarray(xs).reshape(N_CORES, 512, PB).astype(cdt)

    w1f = (w1[:, :, 0, 0] * s1[:, None]).T                    # (512, 256)
    w2f = w2 * s2[:, None, None, None]                        # (256,256,3,3)
    w2t = np.stack(
        [w2f[:, :, dy, dx].T for dy in range(3) for dx in range(3)]
    ).reshape(9 * 256, 256)                                   # (2304, 256)
    w3f = (w3[:, :, 0, 0] * s3[:, None]).T                    # (256, 1024)
    w4f = (w4[:, :, 0, 0] * s4[:, None]).T                    # (512, 1024)

    com = {
        "w1t": np.ascontiguousarray(w1f).astype(cdt),
        "w2t": np.ascontiguousarray(w2t).astype(cdt),
        "w3t": np.ascontiguousarray(w3f).astype(cdt),
        "w4t": np.ascontiguousarray(w4f).astype(cdt),
        "b1p": np.ascontiguousarray(b1.reshape(2, 128).T).astype(np.float32),
        "b2p": np.ascontiguousarray(b2.reshape(2, 128).T).astype(np.float32),
        "b34p": np.ascontiguousarray(
            (b3 + b4).reshape(8, 128).T
        ).astype(np.float32),
    }
    return [{"xs": xs[c], **com} for c in range(N_CORES)]


def _gather(results):
    out = np.empty((64, 1024, HW, HW), np.float32)
    for c, r in enumerate(results):
        o = r["out"].astype(np.float32).reshape(1024, B, HW, HW)
        out[c * B:(c + 1) * B] = o.transpose(1, 0, 2, 3)
    return out


def _get_nc(reps=1, loop_n=0):
    key = ("nc", reps, loop_n)
    if key not in _CACHE:
        _CACHE[key] = _build_nc(reps, loop_n)
    return _CACHE[key]


def _run(in_maps, **kwargs):
    return run_bass_kernel_spmd(
        _get_nc(), in_maps, list(range(N_CORES)), **kwargs
    )


def kernel(**inputs):
    in_maps = _prep(**inputs)
    res = _run(in_maps)
    return _gather(res.results)


def _pjrt_runner(nc, in_maps):
    """Compile nc once; return (run_once, run_batch, results).

    run_once(): one blocking execution. run_batch(n): n pipelined
    executions, blocking at the end; returns elapsed seconds. results:
    first run's outputs as a list of per-core dicts.
    """
    import time

    import jax
    import numpy as np_
    from jax.sharding import Mesh, NamedSharding, PartitionSpec
    from jax.experimental.shard_map import shard_map

    from concourse import bass2jax, mybir as mb

    bass2jax.install_neuronx_cc_hook()
    part_name = nc.partition_id_tensor.name if nc.partition_id_tensor else None
    in_names, out_names, out_avals = [], [], []
    for alloc in nc.m.functions[0].allocations:
        if not isinstance(alloc, mb.MemoryLocationSet):
            continue
        name = alloc.memorylocations[0].name
        if alloc.kind == "ExternalInput":
            if name != part_name:
                in_names.append(name)
        elif alloc.kind == "ExternalOutput":
            out_names.append(name)
            out_avals.append(
                jax.core.ShapedArray(
                    tuple(alloc.tensor_shape), mb.dt.np(alloc.dtype)
                )
            )
    all_names = in_names + out_names + ([part_name] if part_name else [])

    def _body(*args):
        operands = list(args)
        if part_name is not None:
            operands.append(bass2jax.partition_id_tensor())
        outs = bass2jax._bass_exec_p.bind(
            *operands,
            out_avals=tuple(out_avals),
            in_names=tuple(all_names),
            out_names=tuple(out_names),
            lowering_input_output_aliases=(),
            sim_require_finite=False,
            sim_require_nnan=False,
            nc=nc,
        )
        return tuple(outs)

    devices = jax.devices()[:N_CORES]
    mesh = Mesh(np_.asarray(devices), ("core",))
    nspec = len(in_names) + len(out_names)
    sharded = jax.jit(
        shard_map(
            _body,
            mesh=mesh,
            in_specs=(PartitionSpec("core"),) * nspec,
            out_specs=(PartitionSpec("core"),) * len(out_names),
            check_rep=False,
        ),
        keep_unused=True,
    )

    sh = NamedSharding(mesh, PartitionSpec("core"))
    dev_args = [
        jax.device_put(
            np_.concatenate([in_maps[c][n] for c in range(N_CORES)], axis=0), sh
        )
        for n in in_names
    ] + [
        jax.device_put(
            np_.zeros((N_CORES * a.shape[0], *a.shape[1:]), a.dtype), sh
        )
        for a in out_avals
    ]

    outs = jax.block_until_ready(sharded(*dev_args))  # compile + warm

    results = [
        {
            n: np_.asarray(outs[i]).reshape(N_CORES, *out_avals[i].shape)[c]
            for i, n in enumerate(out_names)
        }
        for c in range(N_CORES)
    ]

    def run_once():
        jax.block_until_ready(sharded(*dev_args))

    def run_batch(n):
        t0 = time.monotonic()
        r = None
        for _ in range(n):
            r = sharded(*dev_args)
        jax.block_until_ready(r)
        return time.monotonic() - t0

    return run_once, run_batch, results


def kernel_timed(**inputs):
    """Run + estimate steady-state per-execution device time (ns).

    NTFF profiling is unavailable under this axon client and per-call
    wall time is ~80ms of tunnel overhead, so device time is measured
    with a hardware For_i loop: two NEFFs run the 2-rep body loop_n1 /
    loop_n2 times; the wall-time slope over (loop_n2-loop_n1)*2 reps
    cancels the per-call overhead.  Returns (out, exec_time_ns).
    """
    import time

    import numpy as np_

    n1 = int(os.environ.get("BOT_LOOP_N1", "8"))
    n2 = int(os.environ.get("BOT_LOOP_N2", "64"))
    reps = int(os.environ.get("BOT_LOOP_REPS", "8"))
    trials = int(os.environ.get("BOT_LOOP_TRIALS", "40"))
    in_maps = _prep(**inputs)

    res = _run(in_maps)
    out = _gather(res.results)

    once1, _, _ = _pjrt_runner(_get_nc(reps, loop_n=n1), in_maps)
    once2, _, _ = _pjrt_runner(_get_nc(reps, loop_n=n2), in_maps)

    def s(f):
        t0 = time.monotonic()
        f()
        return time.monotonic() - t0

    for _ in range(2):
        s(once1), s(once2)
    t1s, t2s = [], []
    for _ in range(trials):
        t1s.append(s(once1))
        t2s.append(s(once2))
    t1s, t2s = np_.array(t1s), np_.array(t2s)
    per = (t2s - t1s) / (n2 - n1) / reps * 1e9
    per_rep = int(np_.median(per))
    print(f"[bench] For_i loop slope over {trials} trials: {per_rep} ns "
          f"(iqr {np_.percentile(per, 25):.0f}-{np_.percentile(per, 75):.0f})")
    return out, per_rep
